# revision 2
# baseline (speedup 1.0000x reference)
"""Trainium2 Bass kernel for HSEGNNFlexLayer (GNN message passing).

Wire-optimized SPMD design (8 NeuronCores).  The graded wall-clock is
dominated by host->device transfer over the axon tunnel, so all large
data-dependent staging moves onto the device:

  - x is staged SHARDED in slot order (the per-core block of a
    (core, window, slot) permutation) in bf16 and AllGather'd on device
    into a full DRAM table.  x_j rows are fetched per edge-tile by
    indirect DMA gather from the full table (int32 global-slot grid);
    x_i rows come from the core-local shard with index = window-base +
    uint8 slot (computed on device), so no dst index grid is staged.
  - The scatter one-hot S is gathered per tile from a device-built
    identity table by the same uint8 slot index (255 = zero row kills
    padded edges).
  - Weights/biases stage as one sharded 2-D bf16 image and AllGather.
  - edge_attr / node_attr stage pre-tiled bf16; per-edge extras bf16.

Indirect gathers are issued per tile with [128, 1] offset vectors (one
row per partition) -- the only shape the hardware DGE honors; wider
offset APs silently gather consecutive rows from the first index.

Per-core H2D is ~3.9 MB (~31 MB total vs ~620 MB for the v1 kernel).

Compute pipeline per core: edges dst-partitioned into NWIN windows x 255
slots; TP layers as (lhsT chunks @ flattened W) with attr-weighted
k-sums via per-partition scalar_tensor_tensor chains; scatter-add via
one-hot matmul accumulated in a per-window PSUM bank; node update reads
the core-local shard contiguously.
"""

import numpy as np
import ml_dtypes

import jax
import jax.numpy as jnp
from jax.sharding import Mesh, PartitionSpec, NamedSharding
from jax.experimental.shard_map import shard_map

import concourse.bass as bass
import concourse.mybir as mybir
import concourse.tile as tile
from concourse import bacc
from concourse.bass import IndirectOffsetOnAxis
from concourse.bass2jax import (_bass_exec_p, partition_id_tensor,
                                install_neuronx_cc_hook)
from concourse.masks import make_identity

# Problem constants (hardcoded per contest contract)
N, E, D, A, AM = 50000, 500000, 128, 8, 3
MIN_DIM = 2 * D + AM  # 259
UIN_DIM = D + D + AM  # 259
NCORES = 8
P = 128
KO = A * D  # 1024
SLOTS = 256        # slot space per window (one PSUM bank of f32)
ASSIGN_SLOTS = 255  # slots actually assigned; 255 = pad sentinel
NWIN = 25
BF16 = mybir.dt.bfloat16
F32 = mybir.dt.float32
I32 = mybir.dt.int32
U8 = mybir.dt.uint8
NPBF16 = ml_dtypes.bfloat16

GT = 4  # tiles per DMA group

# Weight-image row layout (rows of 1024 bf16)
W1_R0, W2_R0, W3_R0, W4_R0 = 0, 264, 392, 656
B_R0 = 784          # 4 bias rows (b1..b4), first D entries valid
WIMG_ROWS = 792     # padded to a multiple of NCORES

_cache = {}


# --------------------------------------------------------------------------
# Host-side preparation
# --------------------------------------------------------------------------

def _assign_nodes(dst, n_nodes, nwin):
    """Greedy-pack nodes into NCORES*nwin bins (<=ASSIGN_SLOTS nodes each),
    balancing per-bin edge counts.  Returns (node2bin, node2slot)."""
    import heapq

    counts = np.bincount(dst, minlength=n_nodes)
    order = np.argsort(-counts, kind="stable")
    nbins = NCORES * nwin
    assert nbins * ASSIGN_SLOTS >= n_nodes
    node2bin = np.empty(n_nodes, dtype=np.int32)
    node2slot = np.empty(n_nodes, dtype=np.int32)
    bin_nodes = np.zeros(nbins, dtype=np.int32)
    heap = [(0, b) for b in range(nbins)]
    heapq.heapify(heap)
    for n in order:
        while True:
            c, b = heapq.heappop(heap)
            if bin_nodes[b] < ASSIGN_SLOTS:
                break
        node2bin[n] = b
        node2slot[n] = bin_nodes[b]
        bin_nodes[b] += 1
        heapq.heappush(heap, (c + int(counts[n]), b))
    return node2bin, node2slot


def _prepare(x, edge_attr, node_attr, amf, anf, W1, b1, W2, b2, W3, b3, W4, b4,
             edge_index, n_nodes=N, n_edges=E, nwin=NWIN):
    x = np.asarray(x, dtype=np.float32)
    edge_attr = np.asarray(edge_attr, dtype=np.float32)
    node_attr = np.asarray(node_attr, dtype=np.float32)
    amf = np.asarray(amf, dtype=np.float32)
    anf = np.asarray(anf, dtype=np.float32)
    src = np.asarray(edge_index[0]).astype(np.int32)
    dst = np.asarray(edge_index[1]).astype(np.int32)

    node_slots = nwin * SLOTS  # per-core slot count (incl sentinel slots)

    node2bin, node2slot = _assign_nodes(dst, n_nodes, nwin)
    node_core = node2bin // nwin
    node_win = node2bin % nwin
    node_gslot = node_win * SLOTS + node2slot
    node_tslot = node_core * node_slots + node_gslot  # global table row

    e_bin = node2bin[dst]
    e_order = np.argsort(e_bin, kind="stable")
    e_bin_sorted = e_bin[e_order]
    bin_cnt = np.bincount(e_bin_sorted, minlength=NCORES * nwin)
    T_B = max(1, int(np.ceil(bin_cnt.max() / P)))
    win_cap = T_B * P
    E_pad = nwin * win_cap
    ntiles = nwin * T_B

    bin_starts = np.zeros(NCORES * nwin + 1, dtype=np.int64)
    np.cumsum(bin_cnt, out=bin_starts[1:])
    offs_in_bin = np.arange(len(e_order)) - bin_starts[e_bin_sorted]
    pos = (e_bin_sorted % nwin) * win_cap + offs_in_bin
    core_of_edge = e_bin_sorted // nwin

    # slot-ordered x table, sharded per core
    xperm = np.zeros((NCORES * node_slots, D), dtype=NPBF16)
    xperm[node_tslot] = x[:n_nodes].astype(NPBF16)

    # weight image
    wimg = np.zeros((WIMG_ROWS, KO), dtype=NPBF16)
    wimg[W1_R0:W1_R0 + MIN_DIM] = np.asarray(W1, np.float32).reshape(MIN_DIM, KO).astype(NPBF16)
    wimg[W2_R0:W2_R0 + D] = np.asarray(W2, np.float32).reshape(D, KO).astype(NPBF16)
    wimg[W3_R0:W3_R0 + UIN_DIM] = np.asarray(W3, np.float32).reshape(UIN_DIM, KO).astype(NPBF16)
    wimg[W4_R0:W4_R0 + D] = np.asarray(W4, np.float32).reshape(D, KO).astype(NPBF16)
    for i, b in enumerate((b1, b2, b3, b4)):
        wimg[B_R0 + i, :D] = np.asarray(b, np.float32).astype(NPBF16)
    wsh_rows = WIMG_ROWS // NCORES

    in_maps = []
    slot2node = np.full((NCORES, node_slots), -1, dtype=np.int64)
    nnt = node_slots // P
    for c in range(NCORES):
        mask_c = core_of_edge == c
        pos_c = pos[mask_c]
        eid_c = e_order[mask_c]
        src_c = src[eid_c]
        dst_c = dst[eid_c]

        exj = np.zeros(E_pad, dtype=np.int32)
        eslot = np.full(E_pad, ASSIGN_SLOTS, dtype=np.uint8)
        battr = np.zeros((E_pad, A), dtype=NPBF16)
        amfT = np.zeros((AM, E_pad), dtype=NPBF16)

        exj[pos_c] = node_tslot[src_c]
        eslot[pos_c] = node2slot[dst_c]
        battr[pos_c] = edge_attr[eid_c].astype(NPBF16)
        amfT[:, pos_c] = amf[eid_c].T.astype(NPBF16)

        # (partition, tile) grids: edge e = t*128 + p
        def grid(v):
            return np.ascontiguousarray(v.reshape(ntiles, P).T)

        battrT = np.ascontiguousarray(
            battr.reshape(ntiles, P, A).transpose(1, 0, 2).reshape(P, ntiles * A))

        # node side
        nodes_c = np.nonzero(node_core == c)[0]
        gs = node_gslot[nodes_c]
        slot2node[c, gs] = nodes_c
        nattr = np.zeros((node_slots, A), dtype=NPBF16)
        nattr[gs] = node_attr[nodes_c].astype(NPBF16)
        anfT = np.zeros((AM, node_slots), dtype=NPBF16)
        anfT[:, gs] = anf[nodes_c].T.astype(NPBF16)
        nattrT = np.ascontiguousarray(
            nattr.reshape(nnt, P, A).transpose(1, 0, 2).reshape(P, nnt * A))

        in_maps.append({
            "xsh": np.ascontiguousarray(xperm[c * node_slots:(c + 1) * node_slots]),
            "wsh": np.ascontiguousarray(wimg[c * wsh_rows:(c + 1) * wsh_rows]),
            "exj": grid(exj),
            "eslot8": grid(eslot),
            "battrT": battrT,
            "amfT": np.ascontiguousarray(amfT),
            "nattrT": nattrT,
            "anfT": np.ascontiguousarray(anfT),
        })
    params = dict(T_B=T_B, E_pad=E_pad, nwin=nwin, node_slots=node_slots)
    return in_maps, slot2node, params


# --------------------------------------------------------------------------
# Device kernel builder
# --------------------------------------------------------------------------

def _build(T_B, E_pad, nwin, node_slots):
    nc = bacc.Bacc("TRN2", target_bir_lowering=False, debug=False,
                   num_devices=NCORES)

    wsh_rows = WIMG_ROWS // NCORES
    ntiles = nwin * T_B
    nnt = node_slots // P
    n_tab = NCORES * node_slots

    d_xsh = nc.dram_tensor("xsh", [node_slots, D], BF16, kind="ExternalInput")
    d_wsh = nc.dram_tensor("wsh", [wsh_rows, KO], BF16, kind="ExternalInput")
    d_exj = nc.dram_tensor("exj", [P, ntiles], I32, kind="ExternalInput")
    d_eslot = nc.dram_tensor("eslot8", [P, ntiles], U8, kind="ExternalInput")
    d_battr = nc.dram_tensor("battrT", [P, ntiles * A], BF16, kind="ExternalInput")
    d_amfT = nc.dram_tensor("amfT", [AM, E_pad], BF16, kind="ExternalInput")
    d_nattr = nc.dram_tensor("nattrT", [P, nnt * A], BF16, kind="ExternalInput")
    d_anfT = nc.dram_tensor("anfT", [AM, node_slots], BF16, kind="ExternalInput")
    d_out = nc.dram_tensor("out", [node_slots, D], BF16, kind="ExternalOutput")

    d_xfull = nc.dram_tensor("xfull", [n_tab, D], BF16)
    d_wimg = nc.dram_tensor("wimg", [WIMG_ROWS, KO], BF16)
    d_stab = nc.dram_tensor("stab", [SLOTS, SLOTS], BF16)
    # collectives may not read IO tensors directly -> internal bounces
    d_xsh_b = nc.dram_tensor("xsh_b", [node_slots, D], BF16)
    d_wsh_b = nc.dram_tensor("wsh_b", [wsh_rows, KO], BF16)

    mult = mybir.AluOpType.mult
    add = mybir.AluOpType.add
    silu = mybir.ActivationFunctionType.Silu

    with tile.TileContext(nc) as tc:
        with (
            tc.tile_pool(name="const", bufs=1) as cpool,
            tc.tile_pool(name="ain", bufs=3) as apool,
            tc.tile_pool(name="work", bufs=3) as wpool,
            tc.tile_pool(name="cps", bufs=2, space="PSUM") as cps,
            tc.tile_pool(name="trps", bufs=3, space="PSUM") as trps,
            tc.tile_pool(name="aggps", bufs=1, space="PSUM") as aggps,
        ):
            # ---- collectives: assemble full x table and weight image ----
            nc.gpsimd.dma_start(d_xsh_b.ap(), d_xsh.ap())
            nc.gpsimd.dma_start(d_wsh_b.ap(), d_wsh.ap())
            nc.gpsimd.collective_compute(
                "AllGather", mybir.AluOpType.bypass,
                replica_groups=[list(range(NCORES))],
                ins=[d_xsh_b.ap()], outs=[d_xfull.ap()],
            )
            nc.gpsimd.collective_compute(
                "AllGather", mybir.AluOpType.bypass,
                replica_groups=[list(range(NCORES))],
                ins=[d_wsh_b.ap()], outs=[d_wimg.ap()],
            )

            # ---- constants resident in SBUF ----
            ident = cpool.tile([P, P], BF16, tag="ident", name="ident")
            make_identity(nc, ident[:])

            # S one-hot table: rows 0..254 identity, row 255 zero
            zt = cpool.tile([P, SLOTS], BF16, tag="zt", name="zt")
            nc.vector.memset(zt[:], 0.0)
            for r0 in range(0, SLOTS, P):
                nc.sync.dma_start(d_stab.ap()[r0:r0 + P, :], zt[:])
            for r0 in range(0, SLOTS, P):
                nc.sync.dma_start(d_stab.ap()[r0:r0 + P, r0:r0 + P], ident[:])
            nc.sync.dma_start(d_stab.ap()[SLOTS - 1:SLOTS, :], zt[0:1, :])

            w1c = [cpool.tile([P, KO], BF16, tag="w1c0", name="w1c0"),
                   cpool.tile([P, KO], BF16, tag="w1c1", name="w1c1"),
                   cpool.tile([AM, KO], BF16, tag="w1c2", name="w1c2")]
            nc.sync.dma_start(w1c[0][:], d_wimg.ap()[W1_R0:W1_R0 + P, :])
            nc.sync.dma_start(w1c[1][:], d_wimg.ap()[W1_R0 + P:W1_R0 + 2 * P, :])
            nc.sync.dma_start(w1c[2][:], d_wimg.ap()[W1_R0 + 2 * P:W1_R0 + MIN_DIM, :])
            w2c = cpool.tile([P, KO], BF16, tag="w2c", name="w2c")
            nc.sync.dma_start(w2c[:], d_wimg.ap()[W2_R0:W2_R0 + D, :])
            w3c = [cpool.tile([P, KO], BF16, tag="w3c0", name="w3c0"),
                   cpool.tile([P, KO], BF16, tag="w3c1", name="w3c1"),
                   cpool.tile([AM, KO], BF16, tag="w3c2", name="w3c2")]
            nc.sync.dma_start(w3c[0][:], d_wimg.ap()[W3_R0:W3_R0 + P, :])
            nc.sync.dma_start(w3c[1][:], d_wimg.ap()[W3_R0 + P:W3_R0 + 2 * P, :])
            nc.sync.dma_start(w3c[2][:], d_wimg.ap()[W3_R0 + 2 * P:W3_R0 + UIN_DIM, :])
            w4c = cpool.tile([P, KO], BF16, tag="w4c", name="w4c")
            nc.sync.dma_start(w4c[:], d_wimg.ap()[W4_R0:W4_R0 + D, :])

            # biases: one bf16 row each -> broadcast to 128 partitions -> f32
            btile = []
            for i in range(4):
                brow = cpool.tile([1, D], BF16, tag=f"brow{i}", name=f"brow{i}")
                nc.sync.dma_start(brow[:], d_wimg.ap()[B_R0 + i:B_R0 + i + 1, 0:D])
                bbc = cpool.tile([P, D], BF16, tag=f"bbc{i}", name=f"bbc{i}")
                nc.gpsimd.partition_broadcast(bbc[:], brow[:])
                bt = cpool.tile([P, D], F32, tag=f"bt{i}", name=f"bt{i}")
                nc.vector.tensor_copy(bt[:], bbc[:])
                btile.append(bt)

            aggT = cpool.tile([P, node_slots], BF16, tag="aggT", name="aggT")

            # ---- helpers ----
            def tp_layer(chunks, wchunks, bt_tile, bt_c0, bias_rep, out_tile,
                         do_silu):
                cpsum = cps.tile([P, KO], F32, tag="c", name="c")
                nch = len(chunks)
                for ci in range(nch):
                    for h in range(2):
                        nc.tensor.matmul(
                            cpsum[:, h * 512:(h + 1) * 512],
                            lhsT=chunks[ci],
                            rhs=wchunks[ci][:, h * 512:(h + 1) * 512],
                            start=(ci == 0),
                            stop=(ci == nch - 1),
                        )
                acc = wpool.tile([P, D], F32, tag="acc", name="acc")
                nc.vector.scalar_tensor_tensor(
                    acc[:], cpsum[:, 0:D], bt_tile[:, bt_c0:bt_c0 + 1],
                    bias_rep[:], mult, add)
                for k in range(1, A):
                    nc.vector.scalar_tensor_tensor(
                        acc[:], cpsum[:, k * D:(k + 1) * D],
                        bt_tile[:, bt_c0 + k:bt_c0 + k + 1],
                        acc[:], mult, add)
                if do_silu:
                    nc.scalar.activation(out_tile[:], acc[:], silu)
                else:
                    nc.vector.tensor_copy(out_tile[:], acc[:])

            def transpose_to(src_bf16, tag):
                tps = trps.tile([P, P], BF16, tag="tr", name="tr")
                nc.tensor.transpose(tps[:], src_bf16, ident[:])
                dst = wpool.tile([P, P], BF16, tag=tag, name=tag)
                nc.scalar.copy(dst[:], tps[:])
                return dst

            # ---- edge phase ----
            agg_hold = [None]
            for w in range(nwin):
                for t0 in range(0, T_B, GT):
                    gn = min(GT, T_B - t0)
                    g0 = w * T_B + t0
                    xi4 = apool.tile([P, GT * P], BF16, tag="xi4", name="xi4")
                    xj4 = apool.tile([P, GT * P], BF16, tag="xj4", name="xj4")
                    S4 = apool.tile([P, GT * SLOTS], BF16, tag="S4", name="S4")
                    ixj = apool.tile([P, GT], I32, tag="ixj", name="ixj")
                    isl8 = apool.tile([P, GT], U8, tag="isl8", name="isl8")
                    isl = apool.tile([P, GT], I32, tag="isl", name="isl")
                    ixi = apool.tile([P, GT], I32, tag="ixi", name="ixi")
                    nc.sync.dma_start(ixj[:, :gn], d_exj.ap()[:, g0:g0 + gn])
                    nc.sync.dma_start(isl8[:, :gn], d_eslot.ap()[:, g0:g0 + gn])
                    nc.vector.tensor_copy(isl[:, :gn], isl8[:, :gn])
                    nc.vector.tensor_scalar_add(ixi[:, :gn], isl[:, :gn],
                                                w * SLOTS)
                    am4 = apool.tile([AM, GT * P], BF16, tag="am4", name="am4")
                    nc.sync.dma_start(am4[:, :gn * P],
                                      d_amfT.ap()[:, g0 * P:(g0 + gn) * P])
                    bt4_bf = apool.tile([P, GT * A], BF16, tag="bt4b", name="bt4b")
                    nc.sync.dma_start(bt4_bf[:, :gn * A],
                                      d_battr.ap()[:, g0 * A:(g0 + gn) * A])
                    bt4 = apool.tile([P, GT * A], F32, tag="bt4", name="bt4")
                    nc.vector.tensor_copy(bt4[:, :gn * A], bt4_bf[:, :gn * A])

                    for j in range(gn):
                        tw = t0 + j
                        nc.gpsimd.indirect_dma_start(
                            out=xi4[:, j * P:(j + 1) * P], out_offset=None,
                            in_=d_xsh_b[:], in_offset=IndirectOffsetOnAxis(
                                ap=ixi[:, j:j + 1], axis=0))
                        nc.gpsimd.indirect_dma_start(
                            out=xj4[:, j * P:(j + 1) * P], out_offset=None,
                            in_=d_xfull[:], in_offset=IndirectOffsetOnAxis(
                                ap=ixj[:, j:j + 1], axis=0))
                        nc.gpsimd.indirect_dma_start(
                            out=S4[:, j * SLOTS:(j + 1) * SLOTS],
                            out_offset=None,
                            in_=d_stab[:], in_offset=IndirectOffsetOnAxis(
                                ap=isl[:, j:j + 1], axis=0))

                        xiT = transpose_to(xi4[:, j * P:(j + 1) * P], "xiT")
                        xjT = transpose_to(xj4[:, j * P:(j + 1) * P], "xjT")

                        m1 = wpool.tile([P, D], BF16, tag="m1", name="m1")
                        tp_layer([xiT[:], xjT[:], am4[:, j * P:(j + 1) * P]],
                                 w1c, bt4, j * A, btile[0], m1, True)
                        m1T = transpose_to(m1[:], "m1T")
                        m2 = wpool.tile([P, D], BF16, tag="m2", name="m2")
                        tp_layer([m1T[:]], [w2c], bt4, j * A, btile[1], m2, True)

                        if tw == 0:
                            agg_hold[0] = aggps.tile([P, SLOTS], F32,
                                                     tag="agg", name="agg")
                        agg_ps = agg_hold[0]
                        nc.tensor.matmul(
                            agg_ps[:],
                            lhsT=m2[:],
                            rhs=S4[:, j * SLOTS:(j + 1) * SLOTS],
                            start=(tw == 0),
                            stop=(tw == T_B - 1),
                        )
                        if tw == T_B - 1:
                            nc.vector.tensor_copy(
                                aggT[:, w * SLOTS:(w + 1) * SLOTS], agg_ps[:])

            # ---- node phase ----
            for g0 in range(0, nnt, GT):
                gn = min(GT, nnt - g0)
                xn4 = apool.tile([P, GT * P], BF16, tag="xi4", name="xi4")
                for j in range(gn):
                    t = g0 + j
                    nc.sync.dma_start(xn4[:, j * P:(j + 1) * P],
                                      d_xsh_b.ap()[t * P:(t + 1) * P, :])
                an4 = apool.tile([AM, GT * P], BF16, tag="am4", name="am4")
                nc.sync.dma_start(an4[:, :gn * P],
                                  d_anfT.ap()[:, g0 * P:(g0 + gn) * P])
                na4_bf = apool.tile([P, GT * A], BF16, tag="bt4b", name="bt4b")
                nc.sync.dma_start(na4_bf[:, :gn * A],
                                  d_nattr.ap()[:, g0 * A:(g0 + gn) * A])
                na4 = apool.tile([P, GT * A], F32, tag="bt4", name="bt4")
                nc.vector.tensor_copy(na4[:, :gn * A], na4_bf[:, :gn * A])

                for j in range(gn):
                    t = g0 + j
                    xnT = transpose_to(xn4[:, j * P:(j + 1) * P], "xiT")
                    u = wpool.tile([P, D], BF16, tag="m1", name="m1")
                    tp_layer([xnT[:], aggT[:, t * P:(t + 1) * P],
                              an4[:, j * P:(j + 1) * P]],
                             w3c, na4, j * A, btile[2], u, True)
                    uT = transpose_to(u[:], "m1T")
                    out_t = wpool.tile([P, D], BF16, tag="outt", name="outt")
                    tp_layer([uT[:]], [w4c], na4, j * A, btile[3], out_t, False)
                    nc.sync.dma_start(
                        d_out.ap()[t * P:(t + 1) * P, :], out_t[:])

    nc.compile()
    return nc


# --------------------------------------------------------------------------
# Entry point
# --------------------------------------------------------------------------

def _make_runner(nc):
    """One-time setup: a reusable jitted executor for nc's NEFF.

    run_bass_kernel_spmd constructs a fresh jax.jit per call, paying a
    ~2s retrace+recompile each time; building the jitted callable once
    and creating the donated output buffers on device (instead of
    uploading 13 MB of host zeros) cuts a warm full-input run to ~1 s.
    """
    install_neuronx_cc_hook()
    partition_name = (nc.partition_id_tensor.name
                      if nc.partition_id_tensor else None)
    in_names, out_names, out_avals = [], [], []
    for alloc in nc.m.functions[0].allocations:
        if not isinstance(alloc, mybir.MemoryLocationSet):
            continue
        name = alloc.memorylocations[0].name
        if alloc.kind == "ExternalInput":
            if name != partition_name:
                in_names.append(name)
        elif alloc.kind == "ExternalOutput":
            out_names.append(name)
            out_avals.append(jax.core.ShapedArray(
                tuple(alloc.tensor_shape), mybir.dt.np(alloc.dtype)))
    n_params = len(in_names)
    all_names = in_names + out_names
    if partition_name:
        all_names.append(partition_name)
    donate = tuple(range(n_params, n_params + len(out_avals)))

    def _body(*args):
        operands = list(args)
        if partition_name:
            operands.append(partition_id_tensor())
        return tuple(_bass_exec_p.bind(
            *operands, out_avals=tuple(out_avals), in_names=tuple(all_names),
            out_names=tuple(out_names), lowering_input_output_aliases=(),
            sim_require_finite=True, sim_require_nnan=True, nc=nc))

    devices = jax.devices()[:NCORES]
    mesh = Mesh(np.asarray(devices), ("core",))
    in_specs = (PartitionSpec("core"),) * (n_params + len(out_avals))
    out_specs = (PartitionSpec("core"),) * len(out_names)
    sharded = jax.jit(
        shard_map(_body, mesh=mesh, in_specs=in_specs, out_specs=out_specs,
                  check_rep=False),
        donate_argnums=donate, keep_unused=True)
    zeros_fn = jax.jit(
        lambda: tuple(jnp.zeros((NCORES * a.shape[0], *a.shape[1:]), a.dtype)
                      for a in out_avals),
        out_shardings=tuple(NamedSharding(mesh, PartitionSpec("core"))
                            for _ in out_avals))

    def run(in_maps):
        concat_in = [
            np.concatenate([np.asarray(m[name]) for m in in_maps], axis=0)
            for name in in_names]
        out_arrs = sharded(*concat_in, *zeros_fn())
        return [
            {name: np.asarray(out_arrs[i]).reshape(
                NCORES, *out_avals[i].shape)[c]
             for i, name in enumerate(out_names)}
            for c in range(NCORES)]

    return run


def kernel(x, edge_attr, node_attr, additional_message_features,
           additional_node_features, W1, b1, W2, b2, W3, b3, W4, b4,
           edge_index, batch=None):
    in_maps, slot2node, params = _prepare(
        x, edge_attr, node_attr, additional_message_features,
        additional_node_features, W1, b1, W2, b2, W3, b3, W4, b4, edge_index)

    key = tuple(sorted(params.items()))
    if key not in _cache:
        nc = _build(**params)
        _cache[key] = (nc, _make_runner(nc))
    nc, run = _cache[key]

    results = run(in_maps)
    kernel.last = (nc, in_maps, run, results)

    out = np.zeros((N, D), dtype=np.float32)
    for c in range(NCORES):
        oc = np.asarray(results[c]["out"], dtype=np.float32)
        mask = slot2node[c] >= 0
        out[slot2node[c][mask]] = oc[mask]
    return out


# revision 3
# speedup vs baseline: 2.9598x; 2.9598x over previous
"""Trainium2 Bass kernel for HSEGNNFlexLayer (GNN message passing).

Wire-optimized SPMD design (8 NeuronCores).  The graded wall-clock is
dominated by host->device transfer over the axon tunnel, so all large
data-dependent staging moves onto the device:

  - x is staged SHARDED in slot order (the per-core block of a
    (core, window, slot) permutation) in bf16 and AllGather'd on device
    into a full DRAM table.  x_j rows are fetched per edge-tile by
    indirect DMA gather from the full table (int32 global-slot grid);
    x_i rows come from the core-local shard with index = window-base +
    uint8 slot (computed on device), so no dst index grid is staged.
  - The scatter one-hot S is gathered per tile from a device-built
    identity table by the same uint8 slot index (255 = zero row kills
    padded edges).
  - Weights/biases stage as one sharded 2-D bf16 image and AllGather.
  - edge_attr / node_attr stage pre-tiled bf16; per-edge extras bf16.

Indirect gathers are issued per tile with [128, 1] offset vectors (one
row per partition) -- the only shape the hardware DGE honors; wider
offset APs silently gather consecutive rows from the first index.

Per-core H2D is ~3.9 MB (~31 MB total vs ~620 MB for the v1 kernel).

Compute pipeline per core: edges dst-partitioned into NWIN windows x 255
slots; TP layers as (lhsT chunks @ flattened W) with attr-weighted
k-sums via per-partition scalar_tensor_tensor chains; scatter-add via
one-hot matmul accumulated in a per-window PSUM bank; node update reads
the core-local shard contiguously.
"""

import numpy as np
import ml_dtypes

import jax
import jax.numpy as jnp
from jax.sharding import Mesh, PartitionSpec, NamedSharding
from jax.experimental.shard_map import shard_map

import concourse.bass as bass
import concourse.mybir as mybir
import concourse.tile as tile
from concourse import bacc
from concourse.bass import IndirectOffsetOnAxis
from concourse.bass2jax import (_bass_exec_p, partition_id_tensor,
                                install_neuronx_cc_hook)
from concourse.masks import make_identity

# Problem constants (hardcoded per contest contract)
N, E, D, A, AM = 50000, 500000, 128, 8, 3
MIN_DIM = 2 * D + AM  # 259
UIN_DIM = D + D + AM  # 259
NCORES = 8
P = 128
KO = A * D  # 1024
SLOTS = 256        # slot space per window (one PSUM bank of f32)
ASSIGN_SLOTS = 255  # slots actually assigned; 255 = pad sentinel
NWIN = 25
BF16 = mybir.dt.bfloat16
F32 = mybir.dt.float32
I32 = mybir.dt.int32
U8 = mybir.dt.uint8
U16 = mybir.dt.uint16
NPBF16 = ml_dtypes.bfloat16

GT = 4  # tiles per DMA group

# Weight-image row layout (rows of 1024 bf16)
W1_R0, W2_R0, W3_R0, W4_R0 = 0, 264, 392, 656
B_R0 = 784          # 4 bias rows (b1..b4), first D entries valid
WIMG_ROWS = 792     # padded to a multiple of NCORES

_cache = {}


# --------------------------------------------------------------------------
# Host-side preparation
# --------------------------------------------------------------------------

def _assign_nodes(dst, n_nodes, nwin):
    """Greedy-pack nodes into NCORES*nwin bins (<=ASSIGN_SLOTS nodes each),
    balancing per-bin edge counts.  Returns (node2bin, node2slot)."""
    import heapq

    counts = np.bincount(dst, minlength=n_nodes)
    order = np.argsort(-counts, kind="stable")
    nbins = NCORES * nwin
    assert nbins * ASSIGN_SLOTS >= n_nodes
    node2bin = np.empty(n_nodes, dtype=np.int32)
    node2slot = np.empty(n_nodes, dtype=np.int32)
    bin_nodes = np.zeros(nbins, dtype=np.int32)
    heap = [(0, b) for b in range(nbins)]
    heapq.heapify(heap)
    for n in order:
        while True:
            c, b = heapq.heappop(heap)
            if bin_nodes[b] < ASSIGN_SLOTS:
                break
        node2bin[n] = b
        node2slot[n] = bin_nodes[b]
        bin_nodes[b] += 1
        heapq.heappush(heap, (c + int(counts[n]), b))
    return node2bin, node2slot


def _prepare(x, edge_attr, node_attr, amf, anf, W1, b1, W2, b2, W3, b3, W4, b4,
             edge_index, n_nodes=N, n_edges=E, nwin=NWIN):
    x = np.asarray(x, dtype=np.float32)
    edge_attr = np.asarray(edge_attr, dtype=np.float32)
    node_attr = np.asarray(node_attr, dtype=np.float32)
    amf = np.asarray(amf, dtype=np.float32)
    anf = np.asarray(anf, dtype=np.float32)
    src = np.asarray(edge_index[0]).astype(np.int32)
    dst = np.asarray(edge_index[1]).astype(np.int32)

    node_slots = nwin * SLOTS  # per-core slot count (incl sentinel slots)

    node2bin, node2slot = _assign_nodes(dst, n_nodes, nwin)
    node_core = node2bin // nwin
    node_win = node2bin % nwin
    node_gslot = node_win * SLOTS + node2slot
    node_tslot = node_core * node_slots + node_gslot  # global table row

    e_bin = node2bin[dst]
    e_order = np.argsort(e_bin, kind="stable")
    e_bin_sorted = e_bin[e_order]
    bin_cnt = np.bincount(e_bin_sorted, minlength=NCORES * nwin)
    T_B = max(1, int(np.ceil(bin_cnt.max() / P)))
    win_cap = T_B * P
    E_pad = nwin * win_cap
    ntiles = nwin * T_B

    bin_starts = np.zeros(NCORES * nwin + 1, dtype=np.int64)
    np.cumsum(bin_cnt, out=bin_starts[1:])
    offs_in_bin = np.arange(len(e_order)) - bin_starts[e_bin_sorted]
    pos = (e_bin_sorted % nwin) * win_cap + offs_in_bin
    core_of_edge = e_bin_sorted // nwin

    # slot-ordered x table, sharded per core
    xperm = np.zeros((NCORES * node_slots, D), dtype=NPBF16)
    xperm[node_tslot] = x[:n_nodes].astype(NPBF16)

    # weight image
    wimg = np.zeros((WIMG_ROWS, KO), dtype=NPBF16)
    wimg[W1_R0:W1_R0 + MIN_DIM] = np.asarray(W1, np.float32).reshape(MIN_DIM, KO).astype(NPBF16)
    wimg[W2_R0:W2_R0 + D] = np.asarray(W2, np.float32).reshape(D, KO).astype(NPBF16)
    wimg[W3_R0:W3_R0 + UIN_DIM] = np.asarray(W3, np.float32).reshape(UIN_DIM, KO).astype(NPBF16)
    wimg[W4_R0:W4_R0 + D] = np.asarray(W4, np.float32).reshape(D, KO).astype(NPBF16)
    for i, b in enumerate((b1, b2, b3, b4)):
        wimg[B_R0 + i, :D] = np.asarray(b, np.float32).astype(NPBF16)
    wsh_rows = WIMG_ROWS // NCORES

    in_maps = []
    slot2node = np.full((NCORES, node_slots), -1, dtype=np.int64)
    nnt = node_slots // P
    for c in range(NCORES):
        mask_c = core_of_edge == c
        pos_c = pos[mask_c]
        eid_c = e_order[mask_c]
        src_c = src[eid_c]
        dst_c = dst[eid_c]

        exj = np.zeros(E_pad, dtype=np.uint16)
        eslot = np.full(E_pad, ASSIGN_SLOTS, dtype=np.uint8)
        battr = np.zeros((E_pad, A), dtype=NPBF16)
        amfT = np.zeros((AM, E_pad), dtype=NPBF16)

        exj[pos_c] = node_tslot[src_c].astype(np.uint16)
        eslot[pos_c] = node2slot[dst_c]
        battr[pos_c] = edge_attr[eid_c].astype(NPBF16)
        amfT[:, pos_c] = amf[eid_c].T.astype(NPBF16)

        # (partition, tile) grids: edge e = t*128 + p
        def grid(v):
            return np.ascontiguousarray(v.reshape(ntiles, P).T)

        battrT = np.ascontiguousarray(
            battr.reshape(ntiles, P, A).transpose(1, 0, 2).reshape(P, ntiles * A))

        # node side
        nodes_c = np.nonzero(node_core == c)[0]
        gs = node_gslot[nodes_c]
        slot2node[c, gs] = nodes_c
        nattr = np.zeros((node_slots, A), dtype=NPBF16)
        nattr[gs] = node_attr[nodes_c].astype(NPBF16)
        anfT = np.zeros((AM, node_slots), dtype=NPBF16)
        anfT[:, gs] = anf[nodes_c].T.astype(NPBF16)
        nattrT = np.ascontiguousarray(
            nattr.reshape(nnt, P, A).transpose(1, 0, 2).reshape(P, nnt * A))

        in_maps.append({
            "xsh": np.ascontiguousarray(xperm[c * node_slots:(c + 1) * node_slots]),
            "wsh": np.ascontiguousarray(wimg[c * wsh_rows:(c + 1) * wsh_rows]),
            "exj16": grid(exj),
            "eslot8": grid(eslot),
            "battrT": battrT,
            "amfT": np.ascontiguousarray(amfT),
            "nattrT": nattrT,
            "anfT": np.ascontiguousarray(anfT),
        })
    params = dict(T_B=T_B, E_pad=E_pad, nwin=nwin, node_slots=node_slots)
    return in_maps, slot2node, params


# --------------------------------------------------------------------------
# Device kernel builder
# --------------------------------------------------------------------------

def _build(T_B, E_pad, nwin, node_slots):
    nc = bacc.Bacc("TRN2", target_bir_lowering=False, debug=False,
                   num_devices=NCORES)

    wsh_rows = WIMG_ROWS // NCORES
    ntiles = nwin * T_B
    nnt = node_slots // P
    n_tab = NCORES * node_slots

    d_xsh = nc.dram_tensor("xsh", [node_slots, D], BF16, kind="ExternalInput")
    d_wsh = nc.dram_tensor("wsh", [wsh_rows, KO], BF16, kind="ExternalInput")
    d_exj = nc.dram_tensor("exj16", [P, ntiles], U16, kind="ExternalInput")
    d_eslot = nc.dram_tensor("eslot8", [P, ntiles], U8, kind="ExternalInput")
    d_battr = nc.dram_tensor("battrT", [P, ntiles * A], BF16, kind="ExternalInput")
    d_amfT = nc.dram_tensor("amfT", [AM, E_pad], BF16, kind="ExternalInput")
    d_nattr = nc.dram_tensor("nattrT", [P, nnt * A], BF16, kind="ExternalInput")
    d_anfT = nc.dram_tensor("anfT", [AM, node_slots], BF16, kind="ExternalInput")
    d_out = nc.dram_tensor("out", [node_slots, D], BF16, kind="ExternalOutput")

    d_xfull = nc.dram_tensor("xfull", [n_tab, D], BF16)
    d_wimg = nc.dram_tensor("wimg", [WIMG_ROWS, KO], BF16)
    d_stab = nc.dram_tensor("stab", [SLOTS, SLOTS], BF16)
    # collectives may not read IO tensors directly -> internal bounces
    d_xsh_b = nc.dram_tensor("xsh_b", [node_slots, D], BF16)
    d_wsh_b = nc.dram_tensor("wsh_b", [wsh_rows, KO], BF16)

    mult = mybir.AluOpType.mult
    add = mybir.AluOpType.add
    silu = mybir.ActivationFunctionType.Silu

    with tile.TileContext(nc) as tc:
        with (
            tc.tile_pool(name="const", bufs=1) as cpool,
            tc.tile_pool(name="ain", bufs=3) as apool,
            tc.tile_pool(name="work", bufs=3) as wpool,
            tc.tile_pool(name="cps", bufs=2, space="PSUM") as cps,
            tc.tile_pool(name="trps", bufs=3, space="PSUM") as trps,
            tc.tile_pool(name="aggps", bufs=1, space="PSUM") as aggps,
        ):
            # ---- collectives: assemble full x table and weight image ----
            nc.gpsimd.dma_start(d_xsh_b.ap(), d_xsh.ap())
            nc.gpsimd.dma_start(d_wsh_b.ap(), d_wsh.ap())
            nc.gpsimd.collective_compute(
                "AllGather", mybir.AluOpType.bypass,
                replica_groups=[list(range(NCORES))],
                ins=[d_xsh_b.ap()], outs=[d_xfull.ap()],
            )
            nc.gpsimd.collective_compute(
                "AllGather", mybir.AluOpType.bypass,
                replica_groups=[list(range(NCORES))],
                ins=[d_wsh_b.ap()], outs=[d_wimg.ap()],
            )

            # ---- constants resident in SBUF ----
            ident = cpool.tile([P, P], BF16, tag="ident", name="ident")
            make_identity(nc, ident[:])

            # S one-hot table: rows 0..254 identity, row 255 zero
            zt = cpool.tile([P, SLOTS], BF16, tag="zt", name="zt")
            nc.vector.memset(zt[:], 0.0)
            for r0 in range(0, SLOTS, P):
                nc.sync.dma_start(d_stab.ap()[r0:r0 + P, :], zt[:])
            for r0 in range(0, SLOTS, P):
                nc.sync.dma_start(d_stab.ap()[r0:r0 + P, r0:r0 + P], ident[:])
            nc.sync.dma_start(d_stab.ap()[SLOTS - 1:SLOTS, :], zt[0:1, :])

            w1c = [cpool.tile([P, KO], BF16, tag="w1c0", name="w1c0"),
                   cpool.tile([P, KO], BF16, tag="w1c1", name="w1c1"),
                   cpool.tile([AM, KO], BF16, tag="w1c2", name="w1c2")]
            nc.sync.dma_start(w1c[0][:], d_wimg.ap()[W1_R0:W1_R0 + P, :])
            nc.sync.dma_start(w1c[1][:], d_wimg.ap()[W1_R0 + P:W1_R0 + 2 * P, :])
            nc.sync.dma_start(w1c[2][:], d_wimg.ap()[W1_R0 + 2 * P:W1_R0 + MIN_DIM, :])
            w2c = cpool.tile([P, KO], BF16, tag="w2c", name="w2c")
            nc.sync.dma_start(w2c[:], d_wimg.ap()[W2_R0:W2_R0 + D, :])
            w3c = [cpool.tile([P, KO], BF16, tag="w3c0", name="w3c0"),
                   cpool.tile([P, KO], BF16, tag="w3c1", name="w3c1"),
                   cpool.tile([AM, KO], BF16, tag="w3c2", name="w3c2")]
            nc.sync.dma_start(w3c[0][:], d_wimg.ap()[W3_R0:W3_R0 + P, :])
            nc.sync.dma_start(w3c[1][:], d_wimg.ap()[W3_R0 + P:W3_R0 + 2 * P, :])
            nc.sync.dma_start(w3c[2][:], d_wimg.ap()[W3_R0 + 2 * P:W3_R0 + UIN_DIM, :])
            w4c = cpool.tile([P, KO], BF16, tag="w4c", name="w4c")
            nc.sync.dma_start(w4c[:], d_wimg.ap()[W4_R0:W4_R0 + D, :])

            # biases: one bf16 row each -> broadcast to 128 partitions -> f32
            btile = []
            for i in range(4):
                brow = cpool.tile([1, D], BF16, tag=f"brow{i}", name=f"brow{i}")
                nc.sync.dma_start(brow[:], d_wimg.ap()[B_R0 + i:B_R0 + i + 1, 0:D])
                bbc = cpool.tile([P, D], BF16, tag=f"bbc{i}", name=f"bbc{i}")
                nc.gpsimd.partition_broadcast(bbc[:], brow[:])
                bt = cpool.tile([P, D], F32, tag=f"bt{i}", name=f"bt{i}")
                nc.vector.tensor_copy(bt[:], bbc[:])
                btile.append(bt)

            aggT = cpool.tile([P, node_slots], BF16, tag="aggT", name="aggT")

            # ---- helpers ----
            def tp_layer(chunks, wchunks, bt_tile, bt_c0, bias_rep, out_tile,
                         do_silu):
                cpsum = cps.tile([P, KO], F32, tag="c", name="c")
                nch = len(chunks)
                for ci in range(nch):
                    for h in range(2):
                        nc.tensor.matmul(
                            cpsum[:, h * 512:(h + 1) * 512],
                            lhsT=chunks[ci],
                            rhs=wchunks[ci][:, h * 512:(h + 1) * 512],
                            start=(ci == 0),
                            stop=(ci == nch - 1),
                        )
                acc = wpool.tile([P, D], F32, tag="acc", name="acc")
                nc.vector.scalar_tensor_tensor(
                    acc[:], cpsum[:, 0:D], bt_tile[:, bt_c0:bt_c0 + 1],
                    bias_rep[:], mult, add)
                for k in range(1, A):
                    nc.vector.scalar_tensor_tensor(
                        acc[:], cpsum[:, k * D:(k + 1) * D],
                        bt_tile[:, bt_c0 + k:bt_c0 + k + 1],
                        acc[:], mult, add)
                if do_silu:
                    nc.scalar.activation(out_tile[:], acc[:], silu)
                else:
                    nc.vector.tensor_copy(out_tile[:], acc[:])

            def transpose_to(src_bf16, tag):
                tps = trps.tile([P, P], BF16, tag="tr", name="tr")
                nc.tensor.transpose(tps[:], src_bf16, ident[:])
                dst = wpool.tile([P, P], BF16, tag=tag, name=tag)
                nc.scalar.copy(dst[:], tps[:])
                return dst

            # ---- edge phase ----
            agg_hold = [None]
            for w in range(nwin):
                for t0 in range(0, T_B, GT):
                    gn = min(GT, T_B - t0)
                    g0 = w * T_B + t0
                    xi4 = apool.tile([P, GT * P], BF16, tag="xi4", name="xi4")
                    xj4 = apool.tile([P, GT * P], BF16, tag="xj4", name="xj4")
                    S4 = apool.tile([P, GT * SLOTS], BF16, tag="S4", name="S4")
                    ixj16 = apool.tile([P, GT], U16, tag="ixj16", name="ixj16")
                    ixj = apool.tile([P, GT], I32, tag="ixj", name="ixj")
                    isl8 = apool.tile([P, GT], U8, tag="isl8", name="isl8")
                    isl = apool.tile([P, GT], I32, tag="isl", name="isl")
                    ixi = apool.tile([P, GT], I32, tag="ixi", name="ixi")
                    nc.sync.dma_start(ixj16[:, :gn], d_exj.ap()[:, g0:g0 + gn])
                    nc.vector.tensor_copy(ixj[:, :gn], ixj16[:, :gn])
                    nc.sync.dma_start(isl8[:, :gn], d_eslot.ap()[:, g0:g0 + gn])
                    nc.vector.tensor_copy(isl[:, :gn], isl8[:, :gn])
                    nc.vector.tensor_scalar_add(ixi[:, :gn], isl[:, :gn],
                                                w * SLOTS)
                    am4 = apool.tile([AM, GT * P], BF16, tag="am4", name="am4")
                    nc.sync.dma_start(am4[:, :gn * P],
                                      d_amfT.ap()[:, g0 * P:(g0 + gn) * P])
                    bt4_bf = apool.tile([P, GT * A], BF16, tag="bt4b", name="bt4b")
                    nc.sync.dma_start(bt4_bf[:, :gn * A],
                                      d_battr.ap()[:, g0 * A:(g0 + gn) * A])
                    bt4 = apool.tile([P, GT * A], F32, tag="bt4", name="bt4")
                    nc.vector.tensor_copy(bt4[:, :gn * A], bt4_bf[:, :gn * A])

                    for j in range(gn):
                        tw = t0 + j
                        nc.gpsimd.indirect_dma_start(
                            out=xi4[:, j * P:(j + 1) * P], out_offset=None,
                            in_=d_xsh_b[:], in_offset=IndirectOffsetOnAxis(
                                ap=ixi[:, j:j + 1], axis=0))
                        nc.gpsimd.indirect_dma_start(
                            out=xj4[:, j * P:(j + 1) * P], out_offset=None,
                            in_=d_xfull[:], in_offset=IndirectOffsetOnAxis(
                                ap=ixj[:, j:j + 1], axis=0))
                        nc.gpsimd.indirect_dma_start(
                            out=S4[:, j * SLOTS:(j + 1) * SLOTS],
                            out_offset=None,
                            in_=d_stab[:], in_offset=IndirectOffsetOnAxis(
                                ap=isl[:, j:j + 1], axis=0))

                        xiT = transpose_to(xi4[:, j * P:(j + 1) * P], "xiT")
                        xjT = transpose_to(xj4[:, j * P:(j + 1) * P], "xjT")

                        m1 = wpool.tile([P, D], BF16, tag="m1", name="m1")
                        tp_layer([xiT[:], xjT[:], am4[:, j * P:(j + 1) * P]],
                                 w1c, bt4, j * A, btile[0], m1, True)
                        m1T = transpose_to(m1[:], "m1T")
                        m2 = wpool.tile([P, D], BF16, tag="m2", name="m2")
                        tp_layer([m1T[:]], [w2c], bt4, j * A, btile[1], m2, True)

                        if tw == 0:
                            agg_hold[0] = aggps.tile([P, SLOTS], F32,
                                                     tag="agg", name="agg")
                        agg_ps = agg_hold[0]
                        nc.tensor.matmul(
                            agg_ps[:],
                            lhsT=m2[:],
                            rhs=S4[:, j * SLOTS:(j + 1) * SLOTS],
                            start=(tw == 0),
                            stop=(tw == T_B - 1),
                        )
                        if tw == T_B - 1:
                            nc.vector.tensor_copy(
                                aggT[:, w * SLOTS:(w + 1) * SLOTS], agg_ps[:])

            # ---- node phase ----
            for g0 in range(0, nnt, GT):
                gn = min(GT, nnt - g0)
                xn4 = apool.tile([P, GT * P], BF16, tag="xi4", name="xi4")
                for j in range(gn):
                    t = g0 + j
                    nc.sync.dma_start(xn4[:, j * P:(j + 1) * P],
                                      d_xsh_b.ap()[t * P:(t + 1) * P, :])
                an4 = apool.tile([AM, GT * P], BF16, tag="am4", name="am4")
                nc.sync.dma_start(an4[:, :gn * P],
                                  d_anfT.ap()[:, g0 * P:(g0 + gn) * P])
                na4_bf = apool.tile([P, GT * A], BF16, tag="bt4b", name="bt4b")
                nc.sync.dma_start(na4_bf[:, :gn * A],
                                  d_nattr.ap()[:, g0 * A:(g0 + gn) * A])
                na4 = apool.tile([P, GT * A], F32, tag="bt4", name="bt4")
                nc.vector.tensor_copy(na4[:, :gn * A], na4_bf[:, :gn * A])

                for j in range(gn):
                    t = g0 + j
                    xnT = transpose_to(xn4[:, j * P:(j + 1) * P], "xiT")
                    u = wpool.tile([P, D], BF16, tag="m1", name="m1")
                    tp_layer([xnT[:], aggT[:, t * P:(t + 1) * P],
                              an4[:, j * P:(j + 1) * P]],
                             w3c, na4, j * A, btile[2], u, True)
                    uT = transpose_to(u[:], "m1T")
                    out_t = wpool.tile([P, D], BF16, tag="outt", name="outt")
                    tp_layer([uT[:]], [w4c], na4, j * A, btile[3], out_t, False)
                    nc.sync.dma_start(
                        d_out.ap()[t * P:(t + 1) * P, :], out_t[:])

    nc.compile()
    return nc


# --------------------------------------------------------------------------
# Entry point
# --------------------------------------------------------------------------

def _make_runner(nc):
    """One-time setup: a reusable jitted executor for nc's NEFF.

    run_bass_kernel_spmd constructs a fresh jax.jit per call, paying a
    ~2s retrace+recompile each time; building the jitted callable once
    and creating the donated output buffers on device (instead of
    uploading 13 MB of host zeros) cuts a warm full-input run to ~1 s.
    """
    install_neuronx_cc_hook()
    partition_name = (nc.partition_id_tensor.name
                      if nc.partition_id_tensor else None)
    in_names, out_names, out_avals = [], [], []
    for alloc in nc.m.functions[0].allocations:
        if not isinstance(alloc, mybir.MemoryLocationSet):
            continue
        name = alloc.memorylocations[0].name
        if alloc.kind == "ExternalInput":
            if name != partition_name:
                in_names.append(name)
        elif alloc.kind == "ExternalOutput":
            out_names.append(name)
            out_avals.append(jax.core.ShapedArray(
                tuple(alloc.tensor_shape), mybir.dt.np(alloc.dtype)))
    n_params = len(in_names)
    all_names = in_names + out_names
    if partition_name:
        all_names.append(partition_name)
    donate = tuple(range(n_params, n_params + len(out_avals)))

    def _body(*args):
        operands = list(args)
        if partition_name:
            operands.append(partition_id_tensor())
        return tuple(_bass_exec_p.bind(
            *operands, out_avals=tuple(out_avals), in_names=tuple(all_names),
            out_names=tuple(out_names), lowering_input_output_aliases=(),
            sim_require_finite=True, sim_require_nnan=True, nc=nc))

    devices = jax.devices()[:NCORES]
    mesh = Mesh(np.asarray(devices), ("core",))
    in_specs = (PartitionSpec("core"),) * (n_params + len(out_avals))
    out_specs = (PartitionSpec("core"),) * len(out_names)
    sharded = jax.jit(
        shard_map(_body, mesh=mesh, in_specs=in_specs, out_specs=out_specs,
                  check_rep=False),
        donate_argnums=donate, keep_unused=True)
    zeros_fn = jax.jit(
        lambda: tuple(jnp.zeros((NCORES * a.shape[0], *a.shape[1:]), a.dtype)
                      for a in out_avals),
        out_shardings=tuple(NamedSharding(mesh, PartitionSpec("core"))
                            for _ in out_avals))

    def run(in_maps):
        concat_in = [
            np.concatenate([np.asarray(m[name]) for m in in_maps], axis=0)
            for name in in_names]
        out_arrs = sharded(*concat_in, *zeros_fn())
        return [
            {name: np.asarray(out_arrs[i]).reshape(
                NCORES, *out_avals[i].shape)[c]
             for i, name in enumerate(out_names)}
            for c in range(NCORES)]

    return run


def kernel(x, edge_attr, node_attr, additional_message_features,
           additional_node_features, W1, b1, W2, b2, W3, b3, W4, b4,
           edge_index, batch=None):
    in_maps, slot2node, params = _prepare(
        x, edge_attr, node_attr, additional_message_features,
        additional_node_features, W1, b1, W2, b2, W3, b3, W4, b4, edge_index)

    key = tuple(sorted(params.items()))
    if key not in _cache:
        nc = _build(**params)
        _cache[key] = (nc, _make_runner(nc))
    nc, run = _cache[key]

    results = run(in_maps)
    kernel.last = (nc, in_maps, run, results)

    out = np.zeros((N, D), dtype=np.float32)
    for c in range(NCORES):
        oc = np.asarray(results[c]["out"], dtype=np.float32)
        mask = slot2node[c] >= 0
        out[slot2node[c][mask]] = oc[mask]
    return out


# revision 4
# speedup vs baseline: 3.1231x; 1.0552x over previous
"""Trainium2 Bass kernel for HSEGNNFlexLayer (GNN message passing).

Wire-optimized SPMD design (8 NeuronCores).  The graded wall-clock is
dominated by host->device transfer over the axon tunnel, so all large
data-dependent staging moves onto the device:

  - x is staged SHARDED in slot order (the per-core block of a
    (core, window, slot) permutation) in bf16 and AllGather'd on device
    into a full DRAM table.  x_j rows are fetched per edge-tile by
    indirect DMA gather from the full table (int32 global-slot grid);
    x_i rows come from the core-local shard with index = window-base +
    uint8 slot (computed on device), so no dst index grid is staged.
  - The scatter one-hot S is gathered per tile from a device-built
    identity table by the same uint8 slot index (255 = zero row kills
    padded edges).
  - Weights/biases stage as one sharded 2-D bf16 image and AllGather.
  - edge_attr / node_attr stage pre-tiled bf16; per-edge extras bf16.

Indirect gathers are issued per tile with [128, 1] offset vectors (one
row per partition) -- the only shape the hardware DGE honors; wider
offset APs silently gather consecutive rows from the first index.

Per-core H2D is ~3.9 MB (~31 MB total vs ~620 MB for the v1 kernel).

Compute pipeline per core: edges dst-partitioned into NWIN windows x 255
slots; TP layers as (lhsT chunks @ flattened W) with attr-weighted
k-sums via per-partition scalar_tensor_tensor chains; scatter-add via
one-hot matmul accumulated in a per-window PSUM bank; node update reads
the core-local shard contiguously.
"""

import numpy as np
import ml_dtypes

import jax
import jax.numpy as jnp
from jax.sharding import Mesh, PartitionSpec, NamedSharding
from jax.experimental.shard_map import shard_map

import concourse.bass as bass
import concourse.mybir as mybir
import concourse.tile as tile
from concourse import bacc
from concourse.bass import IndirectOffsetOnAxis
from concourse.bass2jax import (_bass_exec_p, partition_id_tensor,
                                install_neuronx_cc_hook)
from concourse.masks import make_identity

# Problem constants (hardcoded per contest contract)
N, E, D, A, AM = 50000, 500000, 128, 8, 3
MIN_DIM = 2 * D + AM  # 259
UIN_DIM = D + D + AM  # 259
NCORES = 8
P = 128
KO = A * D  # 1024
SLOTS = 256        # slot space per window (one PSUM bank of f32)
ASSIGN_SLOTS = 255  # slots actually assigned; 255 = pad sentinel
NWIN = 25
BF16 = mybir.dt.bfloat16
F32 = mybir.dt.float32
I32 = mybir.dt.int32
U8 = mybir.dt.uint8
U16 = mybir.dt.uint16
FP8 = mybir.dt.float8e3
NPFP8 = ml_dtypes.float8_e3m4
NPBF16 = ml_dtypes.bfloat16

GT = 4  # tiles per DMA group

# Weight-image row layout (rows of 1024 bf16)
W1_R0, W2_R0, W3_R0, W4_R0 = 0, 264, 392, 656
B_R0 = 784          # 4 bias rows (b1..b4), first D entries valid
WIMG_ROWS = 792     # padded to a multiple of NCORES

_cache = {}


# --------------------------------------------------------------------------
# Host-side preparation
# --------------------------------------------------------------------------

def _assign_nodes(dst, n_nodes, nwin):
    """Greedy-pack nodes into NCORES*nwin bins (<=ASSIGN_SLOTS nodes each),
    balancing per-bin edge counts.  Returns (node2bin, node2slot)."""
    import heapq

    counts = np.bincount(dst, minlength=n_nodes)
    order = np.argsort(-counts, kind="stable")
    nbins = NCORES * nwin
    assert nbins * ASSIGN_SLOTS >= n_nodes
    node2bin = np.empty(n_nodes, dtype=np.int32)
    node2slot = np.empty(n_nodes, dtype=np.int32)
    bin_nodes = np.zeros(nbins, dtype=np.int32)
    heap = [(0, b) for b in range(nbins)]
    heapq.heapify(heap)
    for n in order:
        while True:
            c, b = heapq.heappop(heap)
            if bin_nodes[b] < ASSIGN_SLOTS:
                break
        node2bin[n] = b
        node2slot[n] = bin_nodes[b]
        bin_nodes[b] += 1
        heapq.heappush(heap, (c + int(counts[n]), b))
    return node2bin, node2slot


def _prepare(x, edge_attr, node_attr, amf, anf, W1, b1, W2, b2, W3, b3, W4, b4,
             edge_index, n_nodes=N, n_edges=E, nwin=NWIN):
    x = np.asarray(x, dtype=np.float32)
    edge_attr = np.asarray(edge_attr, dtype=np.float32)
    node_attr = np.asarray(node_attr, dtype=np.float32)
    amf = np.asarray(amf, dtype=np.float32)
    anf = np.asarray(anf, dtype=np.float32)
    src = np.asarray(edge_index[0]).astype(np.int32)
    dst = np.asarray(edge_index[1]).astype(np.int32)

    node_slots = nwin * SLOTS  # per-core slot count (incl sentinel slots)

    node2bin, node2slot = _assign_nodes(dst, n_nodes, nwin)
    node_core = node2bin // nwin
    node_win = node2bin % nwin
    node_gslot = node_win * SLOTS + node2slot
    node_tslot = node_core * node_slots + node_gslot  # global table row

    e_bin = node2bin[dst]
    e_order = np.argsort(e_bin, kind="stable")
    e_bin_sorted = e_bin[e_order]
    bin_cnt = np.bincount(e_bin_sorted, minlength=NCORES * nwin)
    T_B = max(1, int(np.ceil(bin_cnt.max() / P)))
    win_cap = T_B * P
    E_pad = nwin * win_cap
    ntiles = nwin * T_B

    bin_starts = np.zeros(NCORES * nwin + 1, dtype=np.int64)
    np.cumsum(bin_cnt, out=bin_starts[1:])
    offs_in_bin = np.arange(len(e_order)) - bin_starts[e_bin_sorted]
    pos = (e_bin_sorted % nwin) * win_cap + offs_in_bin
    core_of_edge = e_bin_sorted // nwin

    # slot-ordered x table, sharded per core
    xperm = np.zeros((NCORES * node_slots, D), dtype=NPBF16)
    xperm[node_tslot] = x[:n_nodes].astype(NPBF16)

    # weight image
    wimg = np.zeros((WIMG_ROWS, KO), dtype=NPBF16)
    wimg[W1_R0:W1_R0 + MIN_DIM] = np.asarray(W1, np.float32).reshape(MIN_DIM, KO).astype(NPBF16)
    wimg[W2_R0:W2_R0 + D] = np.asarray(W2, np.float32).reshape(D, KO).astype(NPBF16)
    wimg[W3_R0:W3_R0 + UIN_DIM] = np.asarray(W3, np.float32).reshape(UIN_DIM, KO).astype(NPBF16)
    wimg[W4_R0:W4_R0 + D] = np.asarray(W4, np.float32).reshape(D, KO).astype(NPBF16)
    for i, b in enumerate((b1, b2, b3, b4)):
        wimg[B_R0 + i, :D] = np.asarray(b, np.float32).astype(NPBF16)
    wsh_rows = WIMG_ROWS // NCORES

    in_maps = []
    slot2node = np.full((NCORES, node_slots), -1, dtype=np.int64)
    nnt = node_slots // P
    for c in range(NCORES):
        mask_c = core_of_edge == c
        pos_c = pos[mask_c]
        eid_c = e_order[mask_c]
        src_c = src[eid_c]
        dst_c = dst[eid_c]

        exj = np.zeros(E_pad, dtype=np.uint16)
        eslot = np.full(E_pad, ASSIGN_SLOTS, dtype=np.uint8)
        battr = np.zeros((E_pad, A), dtype=NPBF16)
        amfT = np.zeros((AM, E_pad), dtype=NPFP8)

        exj[pos_c] = node_tslot[src_c].astype(np.uint16)
        eslot[pos_c] = node2slot[dst_c]
        battr[pos_c] = edge_attr[eid_c].astype(NPBF16)
        amfT[:, pos_c] = amf[eid_c].T.astype(NPFP8)

        # (partition, tile) grids: edge e = t*128 + p
        def grid(v):
            return np.ascontiguousarray(v.reshape(ntiles, P).T)

        battrT = np.ascontiguousarray(
            battr.reshape(ntiles, P, A).transpose(1, 0, 2).reshape(P, ntiles * A))

        # node side
        nodes_c = np.nonzero(node_core == c)[0]
        gs = node_gslot[nodes_c]
        slot2node[c, gs] = nodes_c
        nattr = np.zeros((node_slots, A), dtype=NPBF16)
        nattr[gs] = node_attr[nodes_c].astype(NPBF16)
        anfT = np.zeros((AM, node_slots), dtype=NPFP8)
        anfT[:, gs] = anf[nodes_c].T.astype(NPFP8)
        nattrT = np.ascontiguousarray(
            nattr.reshape(nnt, P, A).transpose(1, 0, 2).reshape(P, nnt * A))

        in_maps.append({
            "xsh": np.ascontiguousarray(xperm[c * node_slots:(c + 1) * node_slots]),
            "wsh": np.ascontiguousarray(wimg[c * wsh_rows:(c + 1) * wsh_rows]),
            "exj16": grid(exj),
            "eslot8": grid(eslot),
            "battrT": battrT,
            "amfT": np.ascontiguousarray(amfT),
            "nattrT": nattrT,
            "anfT": np.ascontiguousarray(anfT),
        })
    params = dict(T_B=T_B, E_pad=E_pad, nwin=nwin, node_slots=node_slots)
    return in_maps, slot2node, params


# --------------------------------------------------------------------------
# Device kernel builder
# --------------------------------------------------------------------------

def _build(T_B, E_pad, nwin, node_slots):
    nc = bacc.Bacc("TRN2", target_bir_lowering=False, debug=False,
                   num_devices=NCORES)

    wsh_rows = WIMG_ROWS // NCORES
    ntiles = nwin * T_B
    nnt = node_slots // P
    n_tab = NCORES * node_slots

    d_xsh = nc.dram_tensor("xsh", [node_slots, D], BF16, kind="ExternalInput")
    d_wsh = nc.dram_tensor("wsh", [wsh_rows, KO], BF16, kind="ExternalInput")
    d_exj = nc.dram_tensor("exj16", [P, ntiles], U16, kind="ExternalInput")
    d_eslot = nc.dram_tensor("eslot8", [P, ntiles], U8, kind="ExternalInput")
    d_battr = nc.dram_tensor("battrT", [P, ntiles * A], BF16, kind="ExternalInput")
    d_amfT = nc.dram_tensor("amfT", [AM, E_pad], FP8, kind="ExternalInput")
    d_nattr = nc.dram_tensor("nattrT", [P, nnt * A], BF16, kind="ExternalInput")
    d_anfT = nc.dram_tensor("anfT", [AM, node_slots], FP8, kind="ExternalInput")
    d_out = nc.dram_tensor("out", [node_slots, D], BF16, kind="ExternalOutput")

    d_xfull = nc.dram_tensor("xfull", [n_tab, D], BF16)
    d_wimg = nc.dram_tensor("wimg", [WIMG_ROWS, KO], BF16)
    d_stab = nc.dram_tensor("stab", [SLOTS, SLOTS], BF16)
    # collectives may not read IO tensors directly -> internal bounces
    d_xsh_b = nc.dram_tensor("xsh_b", [node_slots, D], BF16)
    d_wsh_b = nc.dram_tensor("wsh_b", [wsh_rows, KO], BF16)

    mult = mybir.AluOpType.mult
    add = mybir.AluOpType.add
    silu = mybir.ActivationFunctionType.Silu

    with tile.TileContext(nc) as tc:
        with (
            tc.tile_pool(name="const", bufs=1) as cpool,
            tc.tile_pool(name="ain", bufs=3) as apool,
            tc.tile_pool(name="work", bufs=3) as wpool,
            tc.tile_pool(name="cps", bufs=2, space="PSUM") as cps,
            tc.tile_pool(name="trps", bufs=3, space="PSUM") as trps,
            tc.tile_pool(name="aggps", bufs=1, space="PSUM") as aggps,
        ):
            # ---- collectives: assemble full x table and weight image ----
            nc.gpsimd.dma_start(d_xsh_b.ap(), d_xsh.ap())
            nc.gpsimd.dma_start(d_wsh_b.ap(), d_wsh.ap())
            nc.gpsimd.collective_compute(
                "AllGather", mybir.AluOpType.bypass,
                replica_groups=[list(range(NCORES))],
                ins=[d_xsh_b.ap()], outs=[d_xfull.ap()],
            )
            nc.gpsimd.collective_compute(
                "AllGather", mybir.AluOpType.bypass,
                replica_groups=[list(range(NCORES))],
                ins=[d_wsh_b.ap()], outs=[d_wimg.ap()],
            )

            # ---- constants resident in SBUF ----
            ident = cpool.tile([P, P], BF16, tag="ident", name="ident")
            make_identity(nc, ident[:])

            # S one-hot table: rows 0..254 identity, row 255 zero
            zt = cpool.tile([P, SLOTS], BF16, tag="zt", name="zt")
            nc.vector.memset(zt[:], 0.0)
            for r0 in range(0, SLOTS, P):
                nc.sync.dma_start(d_stab.ap()[r0:r0 + P, :], zt[:])
            for r0 in range(0, SLOTS, P):
                nc.sync.dma_start(d_stab.ap()[r0:r0 + P, r0:r0 + P], ident[:])
            nc.sync.dma_start(d_stab.ap()[SLOTS - 1:SLOTS, :], zt[0:1, :])

            w1c = [cpool.tile([P, KO], BF16, tag="w1c0", name="w1c0"),
                   cpool.tile([P, KO], BF16, tag="w1c1", name="w1c1"),
                   cpool.tile([AM, KO], BF16, tag="w1c2", name="w1c2")]
            nc.sync.dma_start(w1c[0][:], d_wimg.ap()[W1_R0:W1_R0 + P, :])
            nc.sync.dma_start(w1c[1][:], d_wimg.ap()[W1_R0 + P:W1_R0 + 2 * P, :])
            nc.sync.dma_start(w1c[2][:], d_wimg.ap()[W1_R0 + 2 * P:W1_R0 + MIN_DIM, :])
            w2c = cpool.tile([P, KO], BF16, tag="w2c", name="w2c")
            nc.sync.dma_start(w2c[:], d_wimg.ap()[W2_R0:W2_R0 + D, :])
            w3c = [cpool.tile([P, KO], BF16, tag="w3c0", name="w3c0"),
                   cpool.tile([P, KO], BF16, tag="w3c1", name="w3c1"),
                   cpool.tile([AM, KO], BF16, tag="w3c2", name="w3c2")]
            nc.sync.dma_start(w3c[0][:], d_wimg.ap()[W3_R0:W3_R0 + P, :])
            nc.sync.dma_start(w3c[1][:], d_wimg.ap()[W3_R0 + P:W3_R0 + 2 * P, :])
            nc.sync.dma_start(w3c[2][:], d_wimg.ap()[W3_R0 + 2 * P:W3_R0 + UIN_DIM, :])
            w4c = cpool.tile([P, KO], BF16, tag="w4c", name="w4c")
            nc.sync.dma_start(w4c[:], d_wimg.ap()[W4_R0:W4_R0 + D, :])

            # biases: one bf16 row each -> broadcast to 128 partitions -> f32
            btile = []
            for i in range(4):
                brow = cpool.tile([1, D], BF16, tag=f"brow{i}", name=f"brow{i}")
                nc.sync.dma_start(brow[:], d_wimg.ap()[B_R0 + i:B_R0 + i + 1, 0:D])
                bbc = cpool.tile([P, D], BF16, tag=f"bbc{i}", name=f"bbc{i}")
                nc.gpsimd.partition_broadcast(bbc[:], brow[:])
                bt = cpool.tile([P, D], F32, tag=f"bt{i}", name=f"bt{i}")
                nc.vector.tensor_copy(bt[:], bbc[:])
                btile.append(bt)

            aggT = cpool.tile([P, node_slots], BF16, tag="aggT", name="aggT")

            # ---- helpers ----
            def tp_layer(chunks, wchunks, bt_tile, bt_c0, bias_rep, out_tile,
                         do_silu):
                cpsum = cps.tile([P, KO], F32, tag="c", name="c")
                nch = len(chunks)
                for ci in range(nch):
                    for h in range(2):
                        nc.tensor.matmul(
                            cpsum[:, h * 512:(h + 1) * 512],
                            lhsT=chunks[ci],
                            rhs=wchunks[ci][:, h * 512:(h + 1) * 512],
                            start=(ci == 0),
                            stop=(ci == nch - 1),
                        )
                acc = wpool.tile([P, D], F32, tag="acc", name="acc")
                nc.vector.scalar_tensor_tensor(
                    acc[:], cpsum[:, 0:D], bt_tile[:, bt_c0:bt_c0 + 1],
                    bias_rep[:], mult, add)
                for k in range(1, A):
                    nc.vector.scalar_tensor_tensor(
                        acc[:], cpsum[:, k * D:(k + 1) * D],
                        bt_tile[:, bt_c0 + k:bt_c0 + k + 1],
                        acc[:], mult, add)
                if do_silu:
                    nc.scalar.activation(out_tile[:], acc[:], silu)
                else:
                    nc.vector.tensor_copy(out_tile[:], acc[:])

            def transpose_to(src_bf16, tag):
                tps = trps.tile([P, P], BF16, tag="tr", name="tr")
                nc.tensor.transpose(tps[:], src_bf16, ident[:])
                dst = wpool.tile([P, P], BF16, tag=tag, name=tag)
                nc.scalar.copy(dst[:], tps[:])
                return dst

            # ---- edge phase ----
            agg_hold = [None]
            for w in range(nwin):
                for t0 in range(0, T_B, GT):
                    gn = min(GT, T_B - t0)
                    g0 = w * T_B + t0
                    xi4 = apool.tile([P, GT * P], BF16, tag="xi4", name="xi4")
                    xj4 = apool.tile([P, GT * P], BF16, tag="xj4", name="xj4")
                    S4 = apool.tile([P, GT * SLOTS], BF16, tag="S4", name="S4")
                    ixj16 = apool.tile([P, GT], U16, tag="ixj16", name="ixj16")
                    ixj = apool.tile([P, GT], I32, tag="ixj", name="ixj")
                    isl8 = apool.tile([P, GT], U8, tag="isl8", name="isl8")
                    isl = apool.tile([P, GT], I32, tag="isl", name="isl")
                    ixi = apool.tile([P, GT], I32, tag="ixi", name="ixi")
                    nc.sync.dma_start(ixj16[:, :gn], d_exj.ap()[:, g0:g0 + gn])
                    nc.vector.tensor_copy(ixj[:, :gn], ixj16[:, :gn])
                    nc.sync.dma_start(isl8[:, :gn], d_eslot.ap()[:, g0:g0 + gn])
                    nc.vector.tensor_copy(isl[:, :gn], isl8[:, :gn])
                    nc.vector.tensor_scalar_add(ixi[:, :gn], isl[:, :gn],
                                                w * SLOTS)
                    am4_8 = apool.tile([AM, GT * P], FP8, tag="am48", name="am48")
                    nc.sync.dma_start(am4_8[:, :gn * P],
                                      d_amfT.ap()[:, g0 * P:(g0 + gn) * P])
                    am4 = apool.tile([AM, GT * P], BF16, tag="am4", name="am4")
                    nc.vector.tensor_copy(am4[:, :gn * P], am4_8[:, :gn * P])
                    bt4_bf = apool.tile([P, GT * A], BF16, tag="bt4b", name="bt4b")
                    nc.sync.dma_start(bt4_bf[:, :gn * A],
                                      d_battr.ap()[:, g0 * A:(g0 + gn) * A])
                    bt4 = apool.tile([P, GT * A], F32, tag="bt4", name="bt4")
                    nc.vector.tensor_copy(bt4[:, :gn * A], bt4_bf[:, :gn * A])

                    for j in range(gn):
                        tw = t0 + j
                        nc.gpsimd.indirect_dma_start(
                            out=xi4[:, j * P:(j + 1) * P], out_offset=None,
                            in_=d_xsh_b[:], in_offset=IndirectOffsetOnAxis(
                                ap=ixi[:, j:j + 1], axis=0))
                        nc.gpsimd.indirect_dma_start(
                            out=xj4[:, j * P:(j + 1) * P], out_offset=None,
                            in_=d_xfull[:], in_offset=IndirectOffsetOnAxis(
                                ap=ixj[:, j:j + 1], axis=0))
                        nc.gpsimd.indirect_dma_start(
                            out=S4[:, j * SLOTS:(j + 1) * SLOTS],
                            out_offset=None,
                            in_=d_stab[:], in_offset=IndirectOffsetOnAxis(
                                ap=isl[:, j:j + 1], axis=0))

                        xiT = transpose_to(xi4[:, j * P:(j + 1) * P], "xiT")
                        xjT = transpose_to(xj4[:, j * P:(j + 1) * P], "xjT")

                        m1 = wpool.tile([P, D], BF16, tag="m1", name="m1")
                        tp_layer([xiT[:], xjT[:], am4[:, j * P:(j + 1) * P]],
                                 w1c, bt4, j * A, btile[0], m1, True)
                        m1T = transpose_to(m1[:], "m1T")
                        m2 = wpool.tile([P, D], BF16, tag="m2", name="m2")
                        tp_layer([m1T[:]], [w2c], bt4, j * A, btile[1], m2, True)

                        if tw == 0:
                            agg_hold[0] = aggps.tile([P, SLOTS], F32,
                                                     tag="agg", name="agg")
                        agg_ps = agg_hold[0]
                        nc.tensor.matmul(
                            agg_ps[:],
                            lhsT=m2[:],
                            rhs=S4[:, j * SLOTS:(j + 1) * SLOTS],
                            start=(tw == 0),
                            stop=(tw == T_B - 1),
                        )
                        if tw == T_B - 1:
                            nc.vector.tensor_copy(
                                aggT[:, w * SLOTS:(w + 1) * SLOTS], agg_ps[:])

            # ---- node phase ----
            for g0 in range(0, nnt, GT):
                gn = min(GT, nnt - g0)
                xn4 = apool.tile([P, GT * P], BF16, tag="xi4", name="xi4")
                for j in range(gn):
                    t = g0 + j
                    nc.sync.dma_start(xn4[:, j * P:(j + 1) * P],
                                      d_xsh_b.ap()[t * P:(t + 1) * P, :])
                an4_8 = apool.tile([AM, GT * P], FP8, tag="am48", name="am48")
                nc.sync.dma_start(an4_8[:, :gn * P],
                                  d_anfT.ap()[:, g0 * P:(g0 + gn) * P])
                an4 = apool.tile([AM, GT * P], BF16, tag="am4", name="am4")
                nc.vector.tensor_copy(an4[:, :gn * P], an4_8[:, :gn * P])
                na4_bf = apool.tile([P, GT * A], BF16, tag="bt4b", name="bt4b")
                nc.sync.dma_start(na4_bf[:, :gn * A],
                                  d_nattr.ap()[:, g0 * A:(g0 + gn) * A])
                na4 = apool.tile([P, GT * A], F32, tag="bt4", name="bt4")
                nc.vector.tensor_copy(na4[:, :gn * A], na4_bf[:, :gn * A])

                for j in range(gn):
                    t = g0 + j
                    xnT = transpose_to(xn4[:, j * P:(j + 1) * P], "xiT")
                    u = wpool.tile([P, D], BF16, tag="m1", name="m1")
                    tp_layer([xnT[:], aggT[:, t * P:(t + 1) * P],
                              an4[:, j * P:(j + 1) * P]],
                             w3c, na4, j * A, btile[2], u, True)
                    uT = transpose_to(u[:], "m1T")
                    out_t = wpool.tile([P, D], BF16, tag="outt", name="outt")
                    tp_layer([uT[:]], [w4c], na4, j * A, btile[3], out_t, False)
                    nc.sync.dma_start(
                        d_out.ap()[t * P:(t + 1) * P, :], out_t[:])

    nc.compile()
    return nc


# --------------------------------------------------------------------------
# Entry point
# --------------------------------------------------------------------------

def _make_runner(nc):
    """One-time setup: a reusable jitted executor for nc's NEFF.

    run_bass_kernel_spmd constructs a fresh jax.jit per call, paying a
    ~2s retrace+recompile each time; building the jitted callable once
    and creating the donated output buffers on device (instead of
    uploading 13 MB of host zeros) cuts a warm full-input run to ~1 s.
    """
    install_neuronx_cc_hook()
    partition_name = (nc.partition_id_tensor.name
                      if nc.partition_id_tensor else None)
    in_names, out_names, out_avals = [], [], []
    for alloc in nc.m.functions[0].allocations:
        if not isinstance(alloc, mybir.MemoryLocationSet):
            continue
        name = alloc.memorylocations[0].name
        if alloc.kind == "ExternalInput":
            if name != partition_name:
                in_names.append(name)
        elif alloc.kind == "ExternalOutput":
            out_names.append(name)
            out_avals.append(jax.core.ShapedArray(
                tuple(alloc.tensor_shape), mybir.dt.np(alloc.dtype)))
    n_params = len(in_names)
    all_names = in_names + out_names
    if partition_name:
        all_names.append(partition_name)
    donate = tuple(range(n_params, n_params + len(out_avals)))

    def _body(*args):
        operands = list(args)
        if partition_name:
            operands.append(partition_id_tensor())
        return tuple(_bass_exec_p.bind(
            *operands, out_avals=tuple(out_avals), in_names=tuple(all_names),
            out_names=tuple(out_names), lowering_input_output_aliases=(),
            sim_require_finite=True, sim_require_nnan=True, nc=nc))

    devices = jax.devices()[:NCORES]
    mesh = Mesh(np.asarray(devices), ("core",))
    in_specs = (PartitionSpec("core"),) * (n_params + len(out_avals))
    out_specs = (PartitionSpec("core"),) * len(out_names)
    sharded = jax.jit(
        shard_map(_body, mesh=mesh, in_specs=in_specs, out_specs=out_specs,
                  check_rep=False),
        donate_argnums=donate, keep_unused=True)
    zeros_fn = jax.jit(
        lambda: tuple(jnp.zeros((NCORES * a.shape[0], *a.shape[1:]), a.dtype)
                      for a in out_avals),
        out_shardings=tuple(NamedSharding(mesh, PartitionSpec("core"))
                            for _ in out_avals))

    def run(in_maps):
        concat_in = [
            np.concatenate([np.asarray(m[name]) for m in in_maps], axis=0)
            for name in in_names]
        out_arrs = sharded(*concat_in, *zeros_fn())
        return [
            {name: np.asarray(out_arrs[i]).reshape(
                NCORES, *out_avals[i].shape)[c]
             for i, name in enumerate(out_names)}
            for c in range(NCORES)]

    return run


def kernel(x, edge_attr, node_attr, additional_message_features,
           additional_node_features, W1, b1, W2, b2, W3, b3, W4, b4,
           edge_index, batch=None):
    in_maps, slot2node, params = _prepare(
        x, edge_attr, node_attr, additional_message_features,
        additional_node_features, W1, b1, W2, b2, W3, b3, W4, b4, edge_index)

    key = tuple(sorted(params.items()))
    if key not in _cache:
        nc = _build(**params)
        _cache[key] = (nc, _make_runner(nc))
    nc, run = _cache[key]

    results = run(in_maps)
    kernel.last = (nc, in_maps, run, results)

    out = np.zeros((N, D), dtype=np.float32)
    for c in range(NCORES):
        oc = np.asarray(results[c]["out"], dtype=np.float32)
        mask = slot2node[c] >= 0
        out[slot2node[c][mask]] = oc[mask]
    return out


# revision 6
# speedup vs baseline: 4.1482x; 1.3282x over previous
"""Trainium2 Bass kernel for HSEGNNFlexLayer (GNN message passing).

Wire-optimized SPMD design (8 NeuronCores).  The graded wall-clock is
dominated by host->device transfer over the axon tunnel, so all large
data-dependent staging moves onto the device:

  - x is staged SHARDED in slot order (the per-core block of a
    (core, window, slot) permutation) in bf16 and AllGather'd on device
    into a full DRAM table.  x_j rows are fetched per edge-tile by
    indirect DMA gather from the full table (int32 global-slot grid);
    x_i rows come from the core-local shard with index = window-base +
    uint8 slot (computed on device), so no dst index grid is staged.
  - The scatter one-hot S is gathered per tile from a device-built
    identity table by the same uint8 slot index (255 = zero row kills
    padded edges).
  - Weights/biases stage as one sharded 2-D bf16 image and AllGather.
  - edge_attr / node_attr stage pre-tiled bf16; per-edge extras bf16.

Indirect gathers are issued per tile with [128, 1] offset vectors (one
row per partition) -- the only shape the hardware DGE honors; wider
offset APs silently gather consecutive rows from the first index.

Per-core H2D is ~3.9 MB (~31 MB total vs ~620 MB for the v1 kernel).

Compute pipeline per core: edges dst-partitioned into NWIN windows x 255
slots; TP layers as (lhsT chunks @ flattened W) with attr-weighted
k-sums via per-partition scalar_tensor_tensor chains; scatter-add via
one-hot matmul accumulated in a per-window PSUM bank; node update reads
the core-local shard contiguously.
"""

import numpy as np
import ml_dtypes

import jax
import jax.numpy as jnp
from jax.sharding import Mesh, PartitionSpec, NamedSharding
from jax.experimental.shard_map import shard_map

import concourse.bass as bass
import concourse.mybir as mybir
import concourse.tile as tile
from concourse import bacc
from concourse.bass import IndirectOffsetOnAxis
from concourse.bass2jax import (_bass_exec_p, partition_id_tensor,
                                install_neuronx_cc_hook)
from concourse.masks import make_identity

# Problem constants (hardcoded per contest contract)
N, E, D, A, AM = 50000, 500000, 128, 8, 3
MIN_DIM = 2 * D + AM  # 259
UIN_DIM = D + D + AM  # 259
NCORES = 8
P = 128
KO = A * D  # 1024
SLOTS = 256        # slot space per window (one PSUM bank of f32)
ASSIGN_SLOTS = 255  # slots actually assigned; 255 = pad sentinel
NWIN = 25
BF16 = mybir.dt.bfloat16
F32 = mybir.dt.float32
I32 = mybir.dt.int32
U8 = mybir.dt.uint8
U16 = mybir.dt.uint16
FP8 = mybir.dt.float8e3
NPFP8 = ml_dtypes.float8_e3m4
I8 = mybir.dt.int8
NPBF16 = ml_dtypes.bfloat16

GT = 4  # tiles per DMA group

# Weight-image row layout (rows of 1024 bf16)
W1_R0, W2_R0, W3_R0, W4_R0 = 0, 264, 392, 656
B_R0 = 784          # 4 bias rows (b1..b4), first D entries valid
WIMG_ROWS = 792     # padded to a multiple of NCORES

_cache = {}


# --------------------------------------------------------------------------
# Host-side preparation
# --------------------------------------------------------------------------

def _assign_nodes(dst, n_nodes, nwin):
    """Greedy-pack nodes into NCORES*nwin bins (<=ASSIGN_SLOTS nodes each),
    balancing per-bin edge counts.  Returns (node2bin, node2slot)."""
    import heapq

    counts = np.bincount(dst, minlength=n_nodes)
    order = np.argsort(-counts, kind="stable")
    nbins = NCORES * nwin
    assert nbins * ASSIGN_SLOTS >= n_nodes
    node2bin = np.empty(n_nodes, dtype=np.int32)
    node2slot = np.empty(n_nodes, dtype=np.int32)
    bin_nodes = np.zeros(nbins, dtype=np.int32)
    heap = [(0, b) for b in range(nbins)]
    heapq.heapify(heap)
    for n in order:
        while True:
            c, b = heapq.heappop(heap)
            if bin_nodes[b] < ASSIGN_SLOTS:
                break
        node2bin[n] = b
        node2slot[n] = bin_nodes[b]
        bin_nodes[b] += 1
        heapq.heappush(heap, (c + int(counts[n]), b))
    return node2bin, node2slot


def _prepare(x, edge_attr, node_attr, amf, anf, W1, b1, W2, b2, W3, b3, W4, b4,
             edge_index, n_nodes=N, n_edges=E, nwin=NWIN):
    x = np.asarray(x, dtype=np.float32)
    edge_attr = np.asarray(edge_attr, dtype=np.float32)
    node_attr = np.asarray(node_attr, dtype=np.float32)
    amf = np.asarray(amf, dtype=np.float32)
    anf = np.asarray(anf, dtype=np.float32)
    src = np.asarray(edge_index[0]).astype(np.int32)
    dst = np.asarray(edge_index[1]).astype(np.int32)

    node_slots = nwin * SLOTS  # per-core slot count (incl sentinel slots)

    node2bin, node2slot = _assign_nodes(dst, n_nodes, nwin)
    node_core = node2bin // nwin
    node_win = node2bin % nwin
    node_gslot = node_win * SLOTS + node2slot
    node_tslot = node_core * node_slots + node_gslot  # global table row

    e_bin = node2bin[dst]
    e_order = np.argsort(e_bin, kind="stable")
    e_bin_sorted = e_bin[e_order]
    bin_cnt = np.bincount(e_bin_sorted, minlength=NCORES * nwin)
    T_B = max(1, int(np.ceil(bin_cnt.max() / P)))
    win_cap = T_B * P
    E_pad = nwin * win_cap
    ntiles = nwin * T_B

    bin_starts = np.zeros(NCORES * nwin + 1, dtype=np.int64)
    np.cumsum(bin_cnt, out=bin_starts[1:])
    offs_in_bin = np.arange(len(e_order)) - bin_starts[e_bin_sorted]
    pos = (e_bin_sorted % nwin) * win_cap + offs_in_bin
    core_of_edge = e_bin_sorted // nwin

    # slot-ordered x table, sharded per core
    xperm = np.zeros((NCORES * node_slots, D), dtype=NPBF16)
    xperm[node_tslot] = x[:n_nodes].astype(NPBF16)

    # int8 quantization of edge_attr: b = q * s_q with the dequant scale
    # folded into W1/W2 (m = sum_k q_k (a @ s_q*W_k) + bias, exactly)
    s_q = float(np.abs(edge_attr).max()) / 127.0
    if s_q == 0.0:
        s_q = 1.0

    # weight image
    wimg = np.zeros((WIMG_ROWS, KO), dtype=NPBF16)
    wimg[W1_R0:W1_R0 + MIN_DIM] = (np.asarray(W1, np.float32).reshape(MIN_DIM, KO) * s_q).astype(NPBF16)
    wimg[W2_R0:W2_R0 + D] = (np.asarray(W2, np.float32).reshape(D, KO) * s_q).astype(NPBF16)
    wimg[W3_R0:W3_R0 + UIN_DIM] = np.asarray(W3, np.float32).reshape(UIN_DIM, KO).astype(NPBF16)
    wimg[W4_R0:W4_R0 + D] = np.asarray(W4, np.float32).reshape(D, KO).astype(NPBF16)
    for i, b in enumerate((b1, b2, b3, b4)):
        wimg[B_R0 + i, :D] = np.asarray(b, np.float32).astype(NPBF16)
    wsh_rows = WIMG_ROWS // NCORES

    in_maps = []
    slot2node = np.full((NCORES, node_slots), -1, dtype=np.int64)
    nnt = node_slots // P
    for c in range(NCORES):
        mask_c = core_of_edge == c
        pos_c = pos[mask_c]
        eid_c = e_order[mask_c]
        src_c = src[eid_c]
        dst_c = dst[eid_c]

        exj = np.zeros(E_pad, dtype=np.uint16)
        eslot = np.full(E_pad, ASSIGN_SLOTS, dtype=np.uint8)
        battr = np.zeros((E_pad, A), dtype=np.int8)
        amfT = np.zeros((AM, E_pad), dtype=NPFP8)

        exj[pos_c] = node_tslot[src_c].astype(np.uint16)
        eslot[pos_c] = node2slot[dst_c]
        battr[pos_c] = np.clip(np.round(edge_attr[eid_c] / s_q),
                               -127, 127).astype(np.int8)
        amfT[:, pos_c] = amf[eid_c].T.astype(NPFP8)

        # (partition, tile) grids: edge e = t*128 + p
        def grid(v):
            return np.ascontiguousarray(v.reshape(ntiles, P).T)

        battrT = np.ascontiguousarray(
            battr.reshape(ntiles, P, A).transpose(1, 0, 2).reshape(P, ntiles * A))

        # node side
        nodes_c = np.nonzero(node_core == c)[0]
        gs = node_gslot[nodes_c]
        slot2node[c, gs] = nodes_c
        nattr = np.zeros((node_slots, A), dtype=NPBF16)
        nattr[gs] = node_attr[nodes_c].astype(NPBF16)
        anfT = np.zeros((AM, node_slots), dtype=NPFP8)
        anfT[:, gs] = anf[nodes_c].T.astype(NPFP8)
        nattrT = np.ascontiguousarray(
            nattr.reshape(nnt, P, A).transpose(1, 0, 2).reshape(P, nnt * A))

        in_maps.append({
            "xsh": np.ascontiguousarray(xperm[c * node_slots:(c + 1) * node_slots]),
            "wsh": np.ascontiguousarray(wimg[c * wsh_rows:(c + 1) * wsh_rows]),
            "exj16": grid(exj),
            "eslot8": grid(eslot),
            "battrT": battrT,
            "amfT": np.ascontiguousarray(amfT),
            "nattrT": nattrT,
            "anfT": np.ascontiguousarray(anfT),
        })
    params = dict(T_B=T_B, E_pad=E_pad, nwin=nwin, node_slots=node_slots)
    return in_maps, slot2node, params


# --------------------------------------------------------------------------
# Device kernel builder
# --------------------------------------------------------------------------

def _build(T_B, E_pad, nwin, node_slots):
    nc = bacc.Bacc("TRN2", target_bir_lowering=False, debug=False,
                   num_devices=NCORES)

    wsh_rows = WIMG_ROWS // NCORES
    ntiles = nwin * T_B
    nnt = node_slots // P
    n_tab = NCORES * node_slots

    d_xsh = nc.dram_tensor("xsh", [node_slots, D], BF16, kind="ExternalInput")
    d_wsh = nc.dram_tensor("wsh", [wsh_rows, KO], BF16, kind="ExternalInput")
    d_exj = nc.dram_tensor("exj16", [P, ntiles], U16, kind="ExternalInput")
    d_eslot = nc.dram_tensor("eslot8", [P, ntiles], U8, kind="ExternalInput")
    d_battr = nc.dram_tensor("battrT", [P, ntiles * A], I8, kind="ExternalInput")
    d_amfT = nc.dram_tensor("amfT", [AM, E_pad], FP8, kind="ExternalInput")
    d_nattr = nc.dram_tensor("nattrT", [P, nnt * A], BF16, kind="ExternalInput")
    d_anfT = nc.dram_tensor("anfT", [AM, node_slots], FP8, kind="ExternalInput")
    d_out = nc.dram_tensor("out", [node_slots, D], BF16, kind="ExternalOutput")

    d_xfull = nc.dram_tensor("xfull", [n_tab, D], BF16)
    d_wimg = nc.dram_tensor("wimg", [WIMG_ROWS, KO], BF16)
    d_stab = nc.dram_tensor("stab", [SLOTS, SLOTS], BF16)
    # collectives may not read IO tensors directly -> internal bounces
    d_xsh_b = nc.dram_tensor("xsh_b", [node_slots, D], BF16)
    d_wsh_b = nc.dram_tensor("wsh_b", [wsh_rows, KO], BF16)

    mult = mybir.AluOpType.mult
    add = mybir.AluOpType.add
    silu = mybir.ActivationFunctionType.Silu

    with tile.TileContext(nc) as tc:
        with (
            tc.tile_pool(name="const", bufs=1) as cpool,
            tc.tile_pool(name="ain", bufs=3) as apool,
            tc.tile_pool(name="work", bufs=3) as wpool,
            tc.tile_pool(name="cps", bufs=2, space="PSUM") as cps,
            tc.tile_pool(name="trps", bufs=3, space="PSUM") as trps,
            tc.tile_pool(name="aggps", bufs=1, space="PSUM") as aggps,
        ):
            # ---- collectives: assemble full x table and weight image ----
            nc.gpsimd.dma_start(d_xsh_b.ap(), d_xsh.ap())
            nc.gpsimd.dma_start(d_wsh_b.ap(), d_wsh.ap())
            nc.gpsimd.collective_compute(
                "AllGather", mybir.AluOpType.bypass,
                replica_groups=[list(range(NCORES))],
                ins=[d_xsh_b.ap()], outs=[d_xfull.ap()],
            )
            nc.gpsimd.collective_compute(
                "AllGather", mybir.AluOpType.bypass,
                replica_groups=[list(range(NCORES))],
                ins=[d_wsh_b.ap()], outs=[d_wimg.ap()],
            )

            # ---- constants resident in SBUF ----
            ident = cpool.tile([P, P], BF16, tag="ident", name="ident")
            make_identity(nc, ident[:])

            # S one-hot table: rows 0..254 identity, row 255 zero
            zt = cpool.tile([P, SLOTS], BF16, tag="zt", name="zt")
            nc.vector.memset(zt[:], 0.0)
            for r0 in range(0, SLOTS, P):
                nc.sync.dma_start(d_stab.ap()[r0:r0 + P, :], zt[:])
            for r0 in range(0, SLOTS, P):
                nc.sync.dma_start(d_stab.ap()[r0:r0 + P, r0:r0 + P], ident[:])
            nc.sync.dma_start(d_stab.ap()[SLOTS - 1:SLOTS, :], zt[0:1, :])

            w1c = [cpool.tile([P, KO], BF16, tag="w1c0", name="w1c0"),
                   cpool.tile([P, KO], BF16, tag="w1c1", name="w1c1"),
                   cpool.tile([AM, KO], BF16, tag="w1c2", name="w1c2")]
            nc.sync.dma_start(w1c[0][:], d_wimg.ap()[W1_R0:W1_R0 + P, :])
            nc.sync.dma_start(w1c[1][:], d_wimg.ap()[W1_R0 + P:W1_R0 + 2 * P, :])
            nc.sync.dma_start(w1c[2][:], d_wimg.ap()[W1_R0 + 2 * P:W1_R0 + MIN_DIM, :])
            w2c = cpool.tile([P, KO], BF16, tag="w2c", name="w2c")
            nc.sync.dma_start(w2c[:], d_wimg.ap()[W2_R0:W2_R0 + D, :])
            w3c = [cpool.tile([P, KO], BF16, tag="w3c0", name="w3c0"),
                   cpool.tile([P, KO], BF16, tag="w3c1", name="w3c1"),
                   cpool.tile([AM, KO], BF16, tag="w3c2", name="w3c2")]
            nc.sync.dma_start(w3c[0][:], d_wimg.ap()[W3_R0:W3_R0 + P, :])
            nc.sync.dma_start(w3c[1][:], d_wimg.ap()[W3_R0 + P:W3_R0 + 2 * P, :])
            nc.sync.dma_start(w3c[2][:], d_wimg.ap()[W3_R0 + 2 * P:W3_R0 + UIN_DIM, :])
            w4c = cpool.tile([P, KO], BF16, tag="w4c", name="w4c")
            nc.sync.dma_start(w4c[:], d_wimg.ap()[W4_R0:W4_R0 + D, :])

            # biases: one bf16 row each -> broadcast to 128 partitions -> f32
            btile = []
            for i in range(4):
                brow = cpool.tile([1, D], BF16, tag=f"brow{i}", name=f"brow{i}")
                nc.sync.dma_start(brow[:], d_wimg.ap()[B_R0 + i:B_R0 + i + 1, 0:D])
                bbc = cpool.tile([P, D], BF16, tag=f"bbc{i}", name=f"bbc{i}")
                nc.gpsimd.partition_broadcast(bbc[:], brow[:])
                bt = cpool.tile([P, D], F32, tag=f"bt{i}", name=f"bt{i}")
                nc.vector.tensor_copy(bt[:], bbc[:])
                btile.append(bt)

            aggT = cpool.tile([P, node_slots], BF16, tag="aggT", name="aggT")

            # ---- helpers ----
            def tp_layer(chunks, wchunks, bt_tile, bt_c0, bias_rep, out_tile,
                         do_silu):
                cpsum = cps.tile([P, KO], F32, tag="c", name="c")
                nch = len(chunks)
                for ci in range(nch):
                    for h in range(2):
                        nc.tensor.matmul(
                            cpsum[:, h * 512:(h + 1) * 512],
                            lhsT=chunks[ci],
                            rhs=wchunks[ci][:, h * 512:(h + 1) * 512],
                            start=(ci == 0),
                            stop=(ci == nch - 1),
                        )
                acc = wpool.tile([P, D], F32, tag="acc", name="acc")
                nc.vector.scalar_tensor_tensor(
                    acc[:], cpsum[:, 0:D], bt_tile[:, bt_c0:bt_c0 + 1],
                    bias_rep[:], mult, add)
                for k in range(1, A):
                    nc.vector.scalar_tensor_tensor(
                        acc[:], cpsum[:, k * D:(k + 1) * D],
                        bt_tile[:, bt_c0 + k:bt_c0 + k + 1],
                        acc[:], mult, add)
                if do_silu:
                    nc.scalar.activation(out_tile[:], acc[:], silu)
                else:
                    nc.vector.tensor_copy(out_tile[:], acc[:])

            def transpose_to(src_bf16, tag):
                tps = trps.tile([P, P], BF16, tag="tr", name="tr")
                nc.tensor.transpose(tps[:], src_bf16, ident[:])
                dst = wpool.tile([P, P], BF16, tag=tag, name=tag)
                nc.scalar.copy(dst[:], tps[:])
                return dst

            # ---- edge phase ----
            agg_hold = [None]
            for w in range(nwin):
                for t0 in range(0, T_B, GT):
                    gn = min(GT, T_B - t0)
                    g0 = w * T_B + t0
                    xi4 = apool.tile([P, GT * P], BF16, tag="xi4", name="xi4")
                    xj4 = apool.tile([P, GT * P], BF16, tag="xj4", name="xj4")
                    S4 = apool.tile([P, GT * SLOTS], BF16, tag="S4", name="S4")
                    ixj16 = apool.tile([P, GT], U16, tag="ixj16", name="ixj16")
                    ixj = apool.tile([P, GT], I32, tag="ixj", name="ixj")
                    isl8 = apool.tile([P, GT], U8, tag="isl8", name="isl8")
                    isl = apool.tile([P, GT], I32, tag="isl", name="isl")
                    ixi = apool.tile([P, GT], I32, tag="ixi", name="ixi")
                    nc.sync.dma_start(ixj16[:, :gn], d_exj.ap()[:, g0:g0 + gn])
                    nc.vector.tensor_copy(ixj[:, :gn], ixj16[:, :gn])
                    nc.sync.dma_start(isl8[:, :gn], d_eslot.ap()[:, g0:g0 + gn])
                    nc.vector.tensor_copy(isl[:, :gn], isl8[:, :gn])
                    nc.vector.tensor_scalar_add(ixi[:, :gn], isl[:, :gn],
                                                w * SLOTS)
                    am4_8 = apool.tile([AM, GT * P], FP8, tag="am48", name="am48")
                    nc.sync.dma_start(am4_8[:, :gn * P],
                                      d_amfT.ap()[:, g0 * P:(g0 + gn) * P])
                    am4 = apool.tile([AM, GT * P], BF16, tag="am4", name="am4")
                    nc.vector.tensor_copy(am4[:, :gn * P], am4_8[:, :gn * P])
                    bt4_bf = apool.tile([P, GT * A], I8, tag="bt4b", name="bt4b")
                    nc.sync.dma_start(bt4_bf[:, :gn * A],
                                      d_battr.ap()[:, g0 * A:(g0 + gn) * A])
                    bt4 = apool.tile([P, GT * A], F32, tag="bt4", name="bt4")
                    nc.vector.tensor_copy(bt4[:, :gn * A], bt4_bf[:, :gn * A])

                    for j in range(gn):
                        tw = t0 + j
                        nc.gpsimd.indirect_dma_start(
                            out=xi4[:, j * P:(j + 1) * P], out_offset=None,
                            in_=d_xsh_b[:], in_offset=IndirectOffsetOnAxis(
                                ap=ixi[:, j:j + 1], axis=0))
                        nc.gpsimd.indirect_dma_start(
                            out=xj4[:, j * P:(j + 1) * P], out_offset=None,
                            in_=d_xfull[:], in_offset=IndirectOffsetOnAxis(
                                ap=ixj[:, j:j + 1], axis=0))
                        nc.gpsimd.indirect_dma_start(
                            out=S4[:, j * SLOTS:(j + 1) * SLOTS],
                            out_offset=None,
                            in_=d_stab[:], in_offset=IndirectOffsetOnAxis(
                                ap=isl[:, j:j + 1], axis=0))

                        xiT = transpose_to(xi4[:, j * P:(j + 1) * P], "xiT")
                        xjT = transpose_to(xj4[:, j * P:(j + 1) * P], "xjT")

                        m1 = wpool.tile([P, D], BF16, tag="m1", name="m1")
                        tp_layer([xiT[:], xjT[:], am4[:, j * P:(j + 1) * P]],
                                 w1c, bt4, j * A, btile[0], m1, True)
                        m1T = transpose_to(m1[:], "m1T")
                        m2 = wpool.tile([P, D], BF16, tag="m2", name="m2")
                        tp_layer([m1T[:]], [w2c], bt4, j * A, btile[1], m2, True)

                        if tw == 0:
                            agg_hold[0] = aggps.tile([P, SLOTS], F32,
                                                     tag="agg", name="agg")
                        agg_ps = agg_hold[0]
                        nc.tensor.matmul(
                            agg_ps[:],
                            lhsT=m2[:],
                            rhs=S4[:, j * SLOTS:(j + 1) * SLOTS],
                            start=(tw == 0),
                            stop=(tw == T_B - 1),
                        )
                        if tw == T_B - 1:
                            nc.vector.tensor_copy(
                                aggT[:, w * SLOTS:(w + 1) * SLOTS], agg_ps[:])

            # ---- node phase ----
            for g0 in range(0, nnt, GT):
                gn = min(GT, nnt - g0)
                xn4 = apool.tile([P, GT * P], BF16, tag="xi4", name="xi4")
                for j in range(gn):
                    t = g0 + j
                    nc.sync.dma_start(xn4[:, j * P:(j + 1) * P],
                                      d_xsh_b.ap()[t * P:(t + 1) * P, :])
                an4_8 = apool.tile([AM, GT * P], FP8, tag="am48", name="am48")
                nc.sync.dma_start(an4_8[:, :gn * P],
                                  d_anfT.ap()[:, g0 * P:(g0 + gn) * P])
                an4 = apool.tile([AM, GT * P], BF16, tag="am4", name="am4")
                nc.vector.tensor_copy(an4[:, :gn * P], an4_8[:, :gn * P])
                na4_bf = apool.tile([P, GT * A], BF16, tag="na4b", name="na4b")
                nc.sync.dma_start(na4_bf[:, :gn * A],
                                  d_nattr.ap()[:, g0 * A:(g0 + gn) * A])
                na4 = apool.tile([P, GT * A], F32, tag="bt4", name="bt4")
                nc.vector.tensor_copy(na4[:, :gn * A], na4_bf[:, :gn * A])

                for j in range(gn):
                    t = g0 + j
                    xnT = transpose_to(xn4[:, j * P:(j + 1) * P], "xiT")
                    u = wpool.tile([P, D], BF16, tag="m1", name="m1")
                    tp_layer([xnT[:], aggT[:, t * P:(t + 1) * P],
                              an4[:, j * P:(j + 1) * P]],
                             w3c, na4, j * A, btile[2], u, True)
                    uT = transpose_to(u[:], "m1T")
                    out_t = wpool.tile([P, D], BF16, tag="outt", name="outt")
                    tp_layer([uT[:]], [w4c], na4, j * A, btile[3], out_t, False)
                    nc.sync.dma_start(
                        d_out.ap()[t * P:(t + 1) * P, :], out_t[:])

    nc.compile()
    return nc


# --------------------------------------------------------------------------
# Entry point
# --------------------------------------------------------------------------

def _make_runner(nc):
    """One-time setup: a reusable jitted executor for nc's NEFF.

    run_bass_kernel_spmd constructs a fresh jax.jit per call, paying a
    ~2s retrace+recompile each time; building the jitted callable once
    and creating the donated output buffers on device (instead of
    uploading 13 MB of host zeros) cuts a warm full-input run to ~1 s.
    """
    install_neuronx_cc_hook()
    partition_name = (nc.partition_id_tensor.name
                      if nc.partition_id_tensor else None)
    in_names, out_names, out_avals = [], [], []
    for alloc in nc.m.functions[0].allocations:
        if not isinstance(alloc, mybir.MemoryLocationSet):
            continue
        name = alloc.memorylocations[0].name
        if alloc.kind == "ExternalInput":
            if name != partition_name:
                in_names.append(name)
        elif alloc.kind == "ExternalOutput":
            out_names.append(name)
            out_avals.append(jax.core.ShapedArray(
                tuple(alloc.tensor_shape), mybir.dt.np(alloc.dtype)))
    n_params = len(in_names)
    all_names = in_names + out_names
    if partition_name:
        all_names.append(partition_name)
    donate = tuple(range(n_params, n_params + len(out_avals)))

    def _body(*args):
        operands = list(args)
        if partition_name:
            operands.append(partition_id_tensor())
        return tuple(_bass_exec_p.bind(
            *operands, out_avals=tuple(out_avals), in_names=tuple(all_names),
            out_names=tuple(out_names), lowering_input_output_aliases=(),
            sim_require_finite=True, sim_require_nnan=True, nc=nc))

    devices = jax.devices()[:NCORES]
    mesh = Mesh(np.asarray(devices), ("core",))
    in_specs = (PartitionSpec("core"),) * (n_params + len(out_avals))
    out_specs = (PartitionSpec("core"),) * len(out_names)
    sharded = jax.jit(
        shard_map(_body, mesh=mesh, in_specs=in_specs, out_specs=out_specs,
                  check_rep=False),
        donate_argnums=donate, keep_unused=True)
    zeros_fn = jax.jit(
        lambda: tuple(jnp.zeros((NCORES * a.shape[0], *a.shape[1:]), a.dtype)
                      for a in out_avals),
        out_shardings=tuple(NamedSharding(mesh, PartitionSpec("core"))
                            for _ in out_avals))

    def run(in_maps):
        concat_in = [
            np.concatenate([np.asarray(m[name]) for m in in_maps], axis=0)
            for name in in_names]
        out_arrs = sharded(*concat_in, *zeros_fn())
        return [
            {name: np.asarray(out_arrs[i]).reshape(
                NCORES, *out_avals[i].shape)[c]
             for i, name in enumerate(out_names)}
            for c in range(NCORES)]

    return run


def kernel(x, edge_attr, node_attr, additional_message_features,
           additional_node_features, W1, b1, W2, b2, W3, b3, W4, b4,
           edge_index, batch=None):
    in_maps, slot2node, params = _prepare(
        x, edge_attr, node_attr, additional_message_features,
        additional_node_features, W1, b1, W2, b2, W3, b3, W4, b4, edge_index)

    key = tuple(sorted(params.items()))
    if key not in _cache:
        nc = _build(**params)
        _cache[key] = (nc, _make_runner(nc))
    nc, run = _cache[key]

    results = run(in_maps)
    kernel.last = (nc, in_maps, run, results)

    out = np.zeros((N, D), dtype=np.float32)
    for c in range(NCORES):
        oc = np.asarray(results[c]["out"], dtype=np.float32)
        mask = slot2node[c] >= 0
        out[slot2node[c][mask]] = oc[mask]
    return out
